# revision 1
# baseline (speedup 1.0000x reference)
"""CoAttentionFusion Trainium2 kernel (8 NeuronCores, SPMD, no collectives).

Sharding: core c = (batch b = c//2, query-half h = c%2). Each core computes
the full module for its 1024 query rows of batch b; K/V projections over the
full T=2048 are recomputed by both cores of a batch pair (21% redundant
compute, zero communication).

On-chip strategy:
  - activations feature-major (x^T: [d, tokens]) so every linear layer is
    lhsT = W (as stored, [din, dout]), rhs = x^T -> y^T, no transposes.
  - attention computed with transposed scores S^T[k, q] = K^T_h . Q_h^T so the
    exp'd probabilities P^T are directly the moving operand of P@V.
  - V produced token-major with a ones-column appended; the P@V accumulation
    then yields O'^T = [rawO^T ; softmax-denominator] in one group.
  - normalization of O via DVE reciprocal + GPSIMD partition_broadcast.
  - LayerNorms run token-major (per-partition stats) on 128-token chunks,
    entering/leaving via PE transposes.
  - SBUF is tight: x / K / V / O are streamed through DRAM scratch in 512-token
    blocks; K/V projections for attention-2 are emitted interleaved with
    attention-1 (and O-proj/LN of stream t with attention-2) to keep PE busy
    while the ACT engine grinds through exp().
All matmuls bf16 with fp32 PSUM accumulation; softmax/LN math in fp32.
"""

import numpy as np

P = 128
D = 1024
T = 2048
TQ = 1024
NH = 16
HD = 64
DT = D // P          # 8 feature tiles
KT = T // P          # 16 key-token tiles
QC = TQ // P         # 8 query-token chunks
NQ = TQ // 512       # 2 query free-dim tiles
EPS = 1e-5

_WNAMES = ["qt", "kf", "vf", "qf", "kt", "vt", "ot", "of"]


def _build_nc():
    import concourse.bass as bass
    import concourse.tile as tile
    from concourse import bacc, mybir
    from concourse.masks import make_identity
    from contextlib import ExitStack

    f32 = mybir.dt.float32
    bf16 = mybir.dt.bfloat16
    AF = mybir.ActivationFunctionType
    ALU = mybir.AluOpType

    nc = bacc.Bacc("TRN2", target_bir_lowering=False, debug=False, num_devices=8)

    # ---------------- DRAM I/O ----------------
    # x arrives pre-blocked/partition-major: [block, p, dt, 512]
    xtT_d = nc.dram_tensor("xtT", [T // 512, P, DT, 512], bf16,
                           kind="ExternalInput")
    xfT_d = nc.dram_tensor("xfT", [T // 512, P, DT, 512], bf16,
                           kind="ExternalInput")
    xtq_d = nc.dram_tensor("xtq", [TQ, D], f32, kind="ExternalInput")
    xfq_d = nc.dram_tensor("xfq", [TQ, D], f32, kind="ExternalInput")
    # weights pre-shuffled partition-major: [p, kt, dout]
    w_d = {}
    b_d = {}
    for n in _WNAMES:
        w_d[n] = nc.dram_tensor(f"w_{n}", [P, DT, D], bf16, kind="ExternalInput")
        b_d[n] = nc.dram_tensor(f"b_{n}", [P, DT], f32, kind="ExternalInput")
    w_d["f1"] = nc.dram_tensor("w_f1", [P, 2 * DT, D], bf16, kind="ExternalInput")
    b_d["f1"] = nc.dram_tensor("b_f1", [P, DT], f32, kind="ExternalInput")
    w_d["f2"] = nc.dram_tensor("w_f2", [P, DT, D], bf16, kind="ExternalInput")
    b_d["f2"] = nc.dram_tensor("b_f2", [P, DT], f32, kind="ExternalInput")
    # vf/vt biases additionally as broadcast-ready rows
    br_d = {}
    for n in ["vf", "vt"]:
        br_d[n] = nc.dram_tensor(f"br_{n}", [1, D], f32, kind="ExternalInput")
    ln_d = {}
    for n in ["lnt_w", "lnt_b", "lnf_w", "lnf_b", "lnu_w", "lnu_b"]:
        ln_d[n] = nc.dram_tensor(n, [D], f32, kind="ExternalInput")
    out_d = nc.dram_tensor("out", [TQ, D], f32, kind="ExternalOutput")

    with tile.TileContext(nc) as tc, ExitStack() as ctx:
        const = ctx.enter_context(tc.tile_pool(name="const", bufs=1))
        wpool = ctx.enter_context(tc.tile_pool(name="wpool", bufs=2))
        res = ctx.enter_context(tc.tile_pool(name="res", bufs=1))
        xs = ctx.enter_context(tc.tile_pool(name="xs", bufs=3))
        kvs = ctx.enter_context(tc.tile_pool(name="kvs", bufs=3))
        ost = ctx.enter_context(tc.tile_pool(name="ost", bufs=2))
        stg = ctx.enter_context(tc.tile_pool(name="stg", bufs=4))
        ppool = ctx.enter_context(tc.tile_pool(name="ppool", bufs=4))
        spool = ctx.enter_context(tc.tile_pool(name="spool", bufs=2))
        lnp = ctx.enter_context(tc.tile_pool(name="lnp", bufs=2))
        rowp = ctx.enter_context(tc.tile_pool(name="rowp", bufs=1))
        dram = ctx.enter_context(tc.tile_pool(name="dram", bufs=1, space="DRAM"))
        ps_acc = ctx.enter_context(tc.tile_pool(name="ps_acc", bufs=2, space="PSUM"))
        ps_o = ctx.enter_context(tc.tile_pool(name="ps_o", bufs=3, space="PSUM"))
        ps_ln = ctx.enter_context(tc.tile_pool(name="ps_ln", bufs=1, space="PSUM"))

        ident = const.tile([P, P], bf16)
        make_identity(nc, ident[:])
        eps_t = const.tile([P, 1], f32, name="eps")
        nc.gpsimd.memset(eps_t[:], EPS)

        bias_col = {}

        def load_bias_cols():
            for n in ["qt", "kf", "qf", "kt", "ot", "of", "f1", "f2"]:
                t = const.tile([P, DT], f32, name=f"bias_{n}")
                nc.sync.dma_start(t[:], b_d[n][:, :])
                bias_col[n] = t

        def row_bcast(dram_t, tag):
            """[1, D] f32 dram row -> [128, D] bf16 broadcast tile."""
            r = rowp.tile([1, D], f32, tag="row")
            nc.sync.dma_start(r[:], dram_t)
            rb = rowp.tile([1, D], bf16, tag="rowb")
            nc.vector.tensor_copy(rb[:], r[:])
            b = rowp.tile([P, D], bf16, tag=tag)
            nc.gpsimd.partition_broadcast(b[:], rb[:])
            return b

        def load_weight(name, kts=None):
            dram_t = w_d[name]
            if kts is None:
                kts = (0, dram_t.shape[1])
            nkt = kts[1] - kts[0]
            t = wpool.tile([P, nkt, D], bf16, tag="w")
            # split per contraction tile so the first matmul can start early
            for kt in range(nkt):
                nc.sync.dma_start(t[:, kt, :], dram_t[:, kts[0] + kt, :])
            return t

        # DRAM scratch
        kf_dr = dram.tile([D, T], bf16, name="kf_dr")
        kt_dr = dram.tile([D, T], bf16, name="kt_dr")
        vf_dr = dram.tile([NH, P, KT, HD + 1], bf16, name="vf_dr")
        vt_dr = dram.tile([NH, P, KT, HD + 1], bf16, name="vt_dr")
        ot_dr = dram.tile([NQ, P, DT, 512], bf16, name="ot_dr")
        of_dr = dram.tile([NQ, P, DT, 512], bf16, name="of_dr")

        # ------------------------------------------------------------------
        # unit builders (each unit = one closure emitting one psum group)
        # ------------------------------------------------------------------
        def x_block_loader(x_dram, n0):
            blk = {}

            def get():
                if "xb" not in blk:
                    xb = xs.tile([P, DT, 512], bf16, tag="xs")
                    nc.sync.dma_start(xb[:], x_dram[n0 // 512])
                    blk["xb"] = xb
                return blk["xb"]

            return get

        def featmaj_units(w_sb, bname, get_rhs, n0, sink, act=None):
            """y^T[dout, n0:n0+512] units; sink(dt, psum_ap) consumes."""
            units = []
            nkt = w_sb.shape[1]
            for dt in range(DT):

                def u(dt=dt):
                    ps = ps_o.tile([P, 512], f32, tag="ops")
                    rhs = get_rhs()
                    for kt in range(nkt):
                        nc.tensor.matmul(
                            ps[:],
                            w_sb[:, kt, dt * P: (dt + 1) * P],
                            rhs[:, kt, :],
                            start=(kt == 0),
                            stop=(kt == nkt - 1),
                        )
                    sink(dt, ps[:])

                units.append(u)
            return units

        def proj_to_dram_sink(bname, k_dr, n0, eng="act"):
            def sink(dt, ps):
                s = stg.tile([P, 512], bf16, tag="stg")
                if eng == "dve":
                    nc.vector.tensor_scalar_add(
                        s[:], ps, bias_col[bname][:, dt: dt + 1]
                    )
                else:
                    nc.scalar.activation(
                        s[:], ps, AF.Identity,
                        bias=bias_col[bname][:, dt: dt + 1],
                    )
                nc.sync.dma_start(k_dr[dt * P: (dt + 1) * P, n0: n0 + 512], s[:])

            return sink

        def proj_to_sbuf_sink(bname, out_sb, n0, eng="act"):
            def sink(dt, ps):
                if eng == "dve":
                    nc.vector.tensor_scalar_add(
                        out_sb[:, dt, n0: n0 + 512], ps,
                        bias_col[bname][:, dt: dt + 1],
                    )
                else:
                    nc.scalar.activation(
                        out_sb[:, dt, n0: n0 + 512],
                        ps,
                        AF.Identity,
                        bias=bias_col[bname][:, dt: dt + 1],
                    )

            return sink

        def v_units(w_sb, vb_bc, get_x, n0, v_dr):
            """token-major V' units for token block n0 (4 chunks x 2 halves)."""
            units = []
            for tci in range(4):
                for no in range(2):

                    def u(tci=tci, no=no):
                        ps = ps_o.tile([P, 512], f32, tag="ops")
                        xb = get_x()
                        for kt in range(DT):
                            nc.tensor.matmul(
                                ps[:],
                                xb[:, kt, tci * P: (tci + 1) * P],
                                w_sb[:, kt, no * 512: (no + 1) * 512],
                                start=(kt == 0),
                                stop=(kt == DT - 1),
                            )
                        s = stg.tile([P, 8, HD + 1], bf16, tag="stg")
                        nc.vector.tensor_add(
                            s[:, :, 0:HD],
                            ps.rearrange("p (h e) -> p h e", h=8),
                            vb_bc[:, no * 512: (no + 1) * 512].rearrange(
                                "p (h e) -> p h e", h=8
                            ),
                        )
                        nc.gpsimd.memset(s[:, :, HD: HD + 1], 1.0)
                        kt_idx = (n0 + tci * P) // P
                        nc.sync.dma_start(
                            v_dr.rearrange("h p kt e -> p h kt e")[
                                :, no * 8: (no + 1) * 8, kt_idx, :
                            ],
                            s[:],
                        )

                    units.append(u)
            return units

        def attention_units(qT, k_dr, v_dr, o_dr):
            """One closure per (qt, head-pair). Streams K/V', writes O^T."""
            units = []
            for qt in range(NQ):
                for hp in range(NH // 2):

                    def u(qt=qt, hp=hp):
                        kS = kvs.tile([P, T], bf16, tag="kS")
                        for half in range(2):
                            nc.sync.dma_start(
                                kS[:, half * TQ: (half + 1) * TQ],
                                k_dr[hp * P: (hp + 1) * P,
                                     half * TQ: (half + 1) * TQ],
                            )
                        vS = []
                        for sub in range(2):
                            v = kvs.tile([P, KT, HD + 1], bf16, tag="vS")
                            src = v_dr[hp * 2 + sub]
                            for half in range(2):
                                nc.sync.dma_start(
                                    v[:, half * 8: (half + 1) * 8, :],
                                    src[:, half * 8: (half + 1) * 8, :],
                                )
                            vS.append(v)
                        o_ps = [
                            ps_o.tile([P, 512], f32, tag="ops", name=f"o{s}")
                            for s in range(2)
                        ]
                        prev = None
                        for pr in range(KT // 2):
                            cur = []
                            for sub in range(2):
                                lo, hi = sub * HD, (sub + 1) * HD
                                s = ps_acc.tile([P, 2, 512], f32, tag="acc")
                                for j in range(2):
                                    kt = 2 * pr + j
                                    nc.tensor.matmul(
                                        s[:, j, :],
                                        kS[lo:hi, kt * P: (kt + 1) * P],
                                        qT[lo:hi, hp, qt * 512: (qt + 1) * 512],
                                        start=True,
                                        stop=True,
                                        tile_position=(lo, 0),
                                    )
                                pT = ppool.tile([P, 2, 512], bf16, tag="pT")
                                nc.scalar.activation(
                                    pT[:], s[:], AF.Exp, scale=1.0 / 8.0
                                )
                                cur.append((sub, pT))
                            # PV for previous pair (skewed to hide exp latency)
                            if prev is not None:
                                for sub, pTp in prev:
                                    for j in range(2):
                                        kt = 2 * (pr - 1) + j
                                        nc.tensor.matmul(
                                            o_ps[sub][0: HD + 1, :],
                                            vS[sub][:, kt, :],
                                            pTp[:, j, :],
                                            start=(kt == 0),
                                            stop=False,
                                        )
                            prev = cur
                        for sub, pTp in prev:
                            for j in range(2):
                                kt = KT - 2 + j
                                nc.tensor.matmul(
                                    o_ps[sub][0: HD + 1, :],
                                    vS[sub][:, kt, :],
                                    pTp[:, j, :],
                                    start=False,
                                    stop=(j == 1),
                                )
                        for sub in range(2):
                            inv = spool.tile([1, 512], f32, tag="inv")
                            nc.vector.reciprocal(inv[:], o_ps[sub][HD: HD + 1, :])
                            bc = spool.tile([HD, 512], f32, tag="bc")
                            nc.gpsimd.partition_broadcast(bc[:], inv[:])
                            s = stg.tile([HD, 512], bf16, tag="stg")
                            nc.vector.tensor_mul(s[:], o_ps[sub][0:HD, :], bc[:])
                            nc.sync.dma_start(
                                o_dr[qt, sub * HD: (sub + 1) * HD, hp, :], s[:]
                            )

                    units.append(u)
            return units

        def oproj_units(w_sb, bname, o_dr, attnT, eng="act"):
            units = []
            loaders = []
            for n0 in range(0, TQ, 512):
                get = {}

                def get_ob(n0=n0, get=get):
                    if "ob" not in get:
                        ob = ost.tile([P, DT, 512], bf16, tag="os")
                        nc.sync.dma_start(ob[:], o_dr[n0 // 512])
                        get["ob"] = ob
                    return get["ob"]

                loaders.append(get_ob)

                for dt in range(DT):

                    def u(dt=dt, n0=n0, get_ob=get_ob):
                        ps = ps_o.tile([P, 512], f32, tag="ops")
                        ob = get_ob()
                        for kt in range(DT):
                            nc.tensor.matmul(
                                ps[:],
                                w_sb[:, kt, dt * P: (dt + 1) * P],
                                ob[:, kt, :],
                                start=(kt == 0),
                                stop=(kt == DT - 1),
                            )
                        if eng == "dve":
                            nc.vector.tensor_scalar_add(
                                attnT[:, dt, n0: n0 + 512], ps[:],
                                bias_col[bname][:, dt: dt + 1],
                            )
                        else:
                            nc.scalar.activation(
                                attnT[:, dt, n0: n0 + 512],
                                ps[:],
                                AF.Identity,
                                bias=bias_col[bname][:, dt: dt + 1],
                            )

                    units.append(u)
            return units, loaders

        def ln_units(inT, resid_dram, w_bc, b_bc, outT, out_dram=None):
            """Token-major LN, one unit per 128-token chunk."""
            units = []
            for qc in range(QC):

                def u(qc=qc):
                    tok = ps_ln.tile([P, D], bf16, tag="lntok")
                    for dt in range(DT):
                        nc.tensor.transpose(
                            tok[:, dt * P: (dt + 1) * P],
                            inT[:, dt, qc * P: (qc + 1) * P],
                            ident[:],
                        )
                    if resid_dram is not None:
                        s = lnp.tile([P, D], f32, tag="lnB")
                        xq = lnp.tile([P, D], f32, tag="lnA")
                        nc.sync.dma_start(
                            xq[:], resid_dram[qc * P: (qc + 1) * P, :]
                        )
                        nc.vector.tensor_add(s[:], xq[:], tok[:])
                    else:
                        s = tok  # stats/normalize read the PSUM tile directly
                    bns = spool.tile([P, 2, 6], f32, tag="bns")
                    nc.vector.bn_stats(bns[:, 0, :], s[:, 0:512])
                    nc.vector.bn_stats(bns[:, 1, :], s[:, 512:D])
                    mv = spool.tile([P, 2], f32, tag="mv")
                    nc.vector.bn_aggr(mv[:], bns[:])
                    std = spool.tile([P, 1], f32, tag="std")
                    nc.scalar.activation(std[:], mv[:, 1:2], AF.Sqrt, bias=eps_t[:])
                    rstd = spool.tile([P, 1], f32, tag="rstd")
                    nc.vector.reciprocal(rstd[:], std[:])
                    t1 = lnp.tile([P, D], f32, tag="lnA")
                    nc.vector.scalar_tensor_tensor(
                        t1[:], s[:], mv[:, 0:1], w_bc[:],
                        op0=ALU.subtract, op1=ALU.mult,
                    )
                    if out_dram is not None:
                        o = lnp.tile([P, D], f32, tag="lnB")
                        nc.vector.scalar_tensor_tensor(
                            o[:], t1[:], rstd[:], b_bc[:],
                            op0=ALU.mult, op1=ALU.add,
                        )
                        nc.sync.dma_start(out_dram[qc * P: (qc + 1) * P, :], o[:])
                    else:
                        nrm = lnp.tile([P, D], bf16, tag="lnnrm")
                        nc.vector.scalar_tensor_tensor(
                            nrm[:], t1[:], rstd[:], b_bc[:],
                            op0=ALU.mult, op1=ALU.add,
                        )
                        ft = ps_ln.tile([P, D], bf16, tag="lntok")
                        for dt in range(DT):
                            nc.tensor.transpose(
                                ft[:, dt * P: (dt + 1) * P],
                                nrm[:, dt * P: (dt + 1) * P],
                                ident[:],
                            )
                        nc.vector.tensor_copy(
                            outT[:, :, qc * P: (qc + 1) * P],
                            ft.rearrange("p (dt c) -> p dt c", dt=DT),
                        )

                units.append(u)
            return units

        def run_interleaved(primary, filler):
            k = 0
            for i, u in enumerate(primary):
                u()
                want = (i + 1) * len(filler) // len(primary)
                while k < want:
                    filler[k]()
                    k += 1
            while k < len(filler):
                filler[k]()
                k += 1

        # ------------------------------------------------------------------
        # program
        # ------------------------------------------------------------------
        # resident activation tiles (slot-shared by tag across phases)
        qT_t = res.tile([P, DT, TQ], bf16, name="qT_t", tag="qTt")
        qT_f = res.tile([P, DT, TQ], bf16, name="qT_f", tag="qTf")

        # Phase 1: Kf/Vf -> dram, Qt -> sbuf
        loaders = [x_block_loader(xfT_d, n0) for n0 in range(0, T, 512)]
        loaders[0]()  # x DMA issued before the weight DMAs (startup latency)
        w_kf = load_weight("kf")
        load_bias_cols()  # after the critical first x/w DMAs
        w_vf = load_weight("vf")
        vb_f = row_bcast(br_d["vf"][:, :], "vbc")
        for bi, n0 in enumerate(range(0, T, 512)):
            get_x = loaders[bi]
            ku = featmaj_units(
                w_kf, "kf", get_x, n0, proj_to_dram_sink("kf", kf_dr, n0)
            )
            vu = v_units(w_vf, vb_f, get_x, n0, vf_dr)
            run_interleaved(ku, vu)
        w_qt = load_weight("qt")
        for n0 in range(0, TQ, 512):
            get_x = x_block_loader(xtT_d, n0)
            for u in featmaj_units(
                w_qt, "qt", get_x, n0, proj_to_sbuf_sink("qt", qT_t, n0)
            ):
                u()
        # Phase 2: attention-1 (streams kf/vf) || first-half Kt/Vt/Qf units;
        # the second halves (head-pairs 4-7) become early attention-2 fillers.
        w_kt = load_weight("kt")
        w_vt = load_weight("vt")
        w_qf = load_weight("qf")
        vb_t = row_bcast(br_d["vt"][:, :], "vbc")
        fillers = []
        fillers2 = []
        for n0 in range(0, T, 512):
            get_x = x_block_loader(xtT_d, n0)
            fillers += featmaj_units(
                w_kt, "kt", get_x, n0,
                proj_to_dram_sink("kt", kt_dr, n0, eng="dve"),
            )
            fillers += v_units(w_vt, vb_t, get_x, n0, vt_dr)
        for n0 in range(0, TQ, 512):
            get_x = x_block_loader(xfT_d, n0)
            fillers += featmaj_units(
                w_qf, "qf", get_x, n0,
                proj_to_sbuf_sink("qf", qT_f, n0, eng="dve"),
            )
        run_interleaved(attention_units(qT_t, kf_dr, vf_dr, ot_dr), fillers)

        # Phase 3: attention-2 || O-proj(t) + LN(t)
        w_ot = load_weight("ot")
        attnT_t = res.tile([P, DT, TQ], bf16, name="attnT_t", tag="big")
        fusedT_t = res.tile([P, DT, TQ], bf16, name="fusedT_t", tag="qTt")
        lnt_wb = row_bcast(ln_d["lnt_w"].rearrange("(a d) -> a d", a=1), "lnw")
        lnt_bb = row_bcast(ln_d["lnt_b"].rearrange("(a d) -> a d", a=1), "lnb")
        oprojA, _ = oproj_units(w_ot, "ot", ot_dr, attnT_t, eng="dve")
        lnA = ln_units(attnT_t, xtq_d, lnt_wb, lnt_bb, fusedT_t)
        # spread PE-rich oproj units across attn-2's tail; LN chunks (PE-poor,
        # long vector chains) slot between them as soon as their deps allow
        fillers2 += oprojA[:8]
        for i in range(4):
            fillers2.append(oprojA[8 + 2 * i])
            fillers2.append(oprojA[9 + 2 * i])
            fillers2.append(lnA[i])
        fillers2 += lnA[4:]

        # O-proj(f) block 0 only needs the qt=0 half of attention-2's output
        # (and attnT_t to be fully consumed) -- run it as late attn-2 filler.
        w_of = load_weight("of")
        attnT_f = res.tile([P, DT, TQ], bf16, name="attnT_f", tag="big")
        fusedT_f = res.tile([P, DT, TQ], bf16, name="fusedT_f", tag="ff")
        lnf_wb = row_bcast(ln_d["lnf_w"].rearrange("(a d) -> a d", a=1), "lnw")
        lnf_bb = row_bcast(ln_d["lnf_b"].rearrange("(a d) -> a d", a=1), "lnb")
        oprojB, oprojB_ld = oproj_units(w_of, "of", of_dr, attnT_f)
        lnB = ln_units(attnT_f, xfq_d, lnf_wb, lnf_bb, fusedT_f)
        run_interleaved(attention_units(qT_f, kt_dr, vt_dr, of_dr), fillers2)

        # Phases 4+5 (zippered): O-proj(f), LN(f), fus1, fus2, LN(fus) are a
        # pipeline over 512-token blocks; interleave so LN vector math hides
        # under the next stage's matmuls.

        w_f1a = load_weight("f1", kts=(0, DT))
        w_f1b = load_weight("f1", kts=(DT, 2 * DT))
        hT = res.tile([P, DT, TQ], bf16, name="hT", tag="qTf")

        def fus1_units(n0):
            units = []
            for dt in range(DT):

                def u(dt=dt, n0=n0):
                    ps = ps_o.tile([P, 512], f32, tag="ops")
                    for kt in range(DT):
                        nc.tensor.matmul(
                            ps[:],
                            w_f1a[:, kt, dt * P: (dt + 1) * P],
                            fusedT_t[:, kt, n0: n0 + 512],
                            start=(kt == 0),
                            stop=False,
                        )
                    for kt in range(DT):
                        nc.tensor.matmul(
                            ps[:],
                            w_f1b[:, kt, dt * P: (dt + 1) * P],
                            fusedT_f[:, kt, n0: n0 + 512],
                            start=False,
                            stop=(kt == DT - 1),
                        )
                    nc.scalar.activation(
                        hT[:, dt, n0: n0 + 512],
                        ps[:],
                        AF.Gelu,
                        bias=bias_col["f1"][:, dt: dt + 1],
                    )

                units.append(u)
            return units

        w_f2 = load_weight("f2")
        o2T = res.tile([P, DT, TQ], bf16, name="o2T", tag="big")
        lnu_wb = row_bcast(ln_d["lnu_w"].rearrange("(a d) -> a d", a=1), "lnw")
        lnu_bb = row_bcast(ln_d["lnu_b"].rearrange("(a d) -> a d", a=1), "lnb")
        lnU = ln_units(o2T, None, lnu_wb, lnu_bb, None, out_dram=out_d)

        def fus2_units(n0):
            return featmaj_units(
                w_f2, "f2", lambda: hT[:, :, n0: n0 + 512], n0,
                proj_to_sbuf_sink("f2", o2T, n0),
            )

        for u in oprojB[:8]:
            u()
        run_interleaved(oprojB[8:], lnB[:4])
        run_interleaved(fus1_units(0), lnB[4:])
        for u in fus1_units(512):
            u()
        for u in fus2_units(0):
            u()
        run_interleaved(fus2_units(512), lnU[:4])
        for u in lnU[4:]:
            u()

    nc.compile()
    return nc


# ---------------------------------------------------------------------------
# host side
# ---------------------------------------------------------------------------
_CACHE = {}


def _get_nc():
    if "nc" not in _CACHE:
        _CACHE["nc"] = _build_nc()
    return _CACHE["nc"]


def _make_in_maps(inputs):
    import ml_dtypes

    bf16 = ml_dtypes.bfloat16

    def wshuf(w):
        # [din, dout] -> partition-major [128, din/128, dout]
        w = np.asarray(w)
        nkt = w.shape[0] // P
        return np.ascontiguousarray(
            w.reshape(nkt, P, w.shape[1]).transpose(1, 0, 2)
        ).astype(bf16)

    def xshuf(xT):
        # [D, T] -> [T/512 blocks, 128, DT, 512]
        return np.ascontiguousarray(
            xT.reshape(DT, P, T // 512, 512).transpose(2, 1, 0, 3)
        ).astype(bf16)

    t = np.asarray(inputs["temporal_tokens"], np.float32)
    f = np.asarray(inputs["feature_tokens"], np.float32)

    def bshuf(b):
        # [D] -> [128, DT] (partition-major, contiguous per partition)
        return np.ascontiguousarray(
            np.asarray(b, np.float32).reshape(DT, P).T)

    shared = {}
    for n in _WNAMES:
        shared[f"w_{n}"] = wshuf(inputs[f"{n}_w"])
        shared[f"b_{n}"] = bshuf(inputs[f"{n}_b"])
    shared["w_f1"] = wshuf(inputs["fus1_w"])
    shared["b_f1"] = bshuf(inputs["fus1_b"])
    shared["w_f2"] = wshuf(inputs["fus2_w"])
    shared["b_f2"] = bshuf(inputs["fus2_b"])
    for n in ["vf", "vt"]:
        shared[f"br_{n}"] = np.ascontiguousarray(
            np.asarray(inputs[f"{n}_b"], np.float32).reshape(1, D))
    for src, dst in [
        ("ln_t_w", "lnt_w"), ("ln_t_b", "lnt_b"),
        ("ln_f_w", "lnf_w"), ("ln_f_b", "lnf_b"),
        ("ln_fus_w", "lnu_w"), ("ln_fus_b", "lnu_b"),
    ]:
        shared[dst] = np.ascontiguousarray(inputs[src], dtype=np.float32)

    in_maps = []
    for c in range(8):
        b, half = divmod(c, 2)
        r0 = half * TQ
        xt = t[b]
        xf = f[b]
        # query rows first, remaining rows after (K/V order is irrelevant)
        perm = np.concatenate([np.arange(r0, T), np.arange(0, r0)])
        m = dict(shared)
        m["xtT"] = xshuf(xt[perm].T)
        m["xfT"] = xshuf(xf[perm].T)
        m["xtq"] = np.ascontiguousarray(xt[r0: r0 + TQ])
        m["xfq"] = np.ascontiguousarray(xf[r0: r0 + TQ])
        in_maps.append(m)
    return in_maps


def kernel(**inputs):
    try:
        import jax

        jax.config.update("jax_compilation_cache_dir", "/tmp/jaxcache")
        jax.config.update("jax_persistent_cache_min_entry_size_bytes", -1)
        jax.config.update("jax_persistent_cache_min_compile_time_secs", 0.0)
    except Exception:
        pass
    from concourse.bass_utils import run_bass_kernel_spmd

    nc = _get_nc()
    in_maps = _make_in_maps(inputs)
    res = run_bass_kernel_spmd(nc, in_maps, list(range(8)))
    B = 4
    out = np.empty((B, T, D), np.float32)
    for c in range(8):
        b, half = divmod(c, 2)
        out[b, half * TQ: (half + 1) * TQ] = res.results[c]["out"]
    return out



# revision 41
# speedup vs baseline: 1.2567x; 1.2567x over previous
"""CoAttentionFusion Trainium2 kernel (8 NeuronCores, SPMD, no collectives).

Sharding: core c = (batch b = c//2, query-half h = c%2). Each core computes
the full module for its 1024 query rows of batch b; K/V projections over the
full T=2048 are recomputed by both cores of a batch pair.

v2: fp8e4m3 + DoubleRow matmuls everywhere.
  - weights pre-scaled x32 into e4m3's normal range, de-scaled (x1/32) for
    free in the bias sinks; activations quantized to e4m3 at each sink.
  - projections: contraction 1024 = 4 DoubleRow pairs (2 k-tiles/matmul).
  - scores: per-head contraction 64 with a zero-interleaved Q layout
    ([Q0 | 0 | Q1 | 0]) so the DoubleRow pair's second subtile multiplies
    zeros: 128 keys x 512 queries per 256-cycle matmul.
  - PV: V' stored [token, kt, 80] (64 feats + ones col + pad to the 16B
    dual-fp8 stride rule); 2 key-tiles per DoubleRow matmul.
  - K/V/O/Q all SBUF-resident in fp8 (no DRAM scratch roundtrips).
  - ACT runs only exp (softmax) + gelu + LN-sqrt; all bias sinks on DVE,
    V sinks and broadcasts on GPSIMD. ACT exp is the critical path.
  - attention emitted at 2-keytile group granularity; projection/LN units
    interleave as PE filler to keep the PE warm under the ACT-bound phases.
"""

import numpy as np

P = 128
D = 1024
T = 2048
TQ = 1024
NH = 16
HD = 64
DT = D // P          # 8 feature tiles
KT = T // P          # 16 key-token tiles
QC = TQ // P         # 8 query-token chunks
NQ = TQ // 512       # 2 query free-dim tiles
VP = 80              # V' padded cols: 64 feats + 1 ones + 15 pad
WS = 32.0            # weight prescale
EPS = 1e-5

_WNAMES = ["qt", "kf", "vf", "qf", "kt", "vt", "ot", "of"]


def _build_nc():
    import concourse.bass as bass
    import concourse.tile as tile
    from concourse import bacc, mybir
    from concourse.masks import make_identity
    from contextlib import ExitStack

    f32 = mybir.dt.float32
    bf16 = mybir.dt.bfloat16
    fp8 = mybir.dt.float8e4
    AF = mybir.ActivationFunctionType
    ALU = mybir.AluOpType
    PM = mybir.MatmulPerfMode

    nc = bacc.Bacc("TRN2", target_bir_lowering=False, debug=False, num_devices=8)

    # ---------------- DRAM I/O ----------------
    # x pre-blocked/partition-major fp8: [block, p, dt, 512]
    xtT_d = nc.dram_tensor("xtT", [T // 512, P, DT, 512], fp8,
                           kind="ExternalInput")
    xfT_d = nc.dram_tensor("xfT", [T // 512, P, DT, 512], fp8,
                           kind="ExternalInput")
    xtq_d = nc.dram_tensor("xtq", [TQ, D], bf16, kind="ExternalInput")
    xfq_d = nc.dram_tensor("xfq", [TQ, D], bf16, kind="ExternalInput")
    # weights pre-shuffled partition-major fp8 (x32): [p, kt, dout]
    w_d = {}
    b_d = {}
    for n in _WNAMES:
        w_d[n] = nc.dram_tensor(f"w_{n}", [P, DT, D], fp8, kind="ExternalInput")
        b_d[n] = nc.dram_tensor(f"b_{n}", [P, DT], f32, kind="ExternalInput")
    w_d["f1"] = nc.dram_tensor("w_f1", [P, 2 * DT, D], fp8, kind="ExternalInput")
    b_d["f1"] = nc.dram_tensor("b_f1", [P, DT], f32, kind="ExternalInput")
    w_d["f2"] = nc.dram_tensor("w_f2", [P, DT, D], fp8, kind="ExternalInput")
    b_d["f2"] = nc.dram_tensor("b_f2", [P, DT], f32, kind="ExternalInput")
    br_d = {}
    for n in ["vf", "vt"]:
        br_d[n] = nc.dram_tensor(f"br_{n}", [1, D], bf16, kind="ExternalInput")
    ln_d = {}
    for n in ["lnt_w", "lnt_b", "lnf_w", "lnf_b", "lnu_w", "lnu_b"]:
        ln_d[n] = nc.dram_tensor(n, [D], bf16, kind="ExternalInput")
    out_d = nc.dram_tensor("out", [TQ, D], f32, kind="ExternalOutput")

    with tile.TileContext(nc) as tc, ExitStack() as ctx:
        const = ctx.enter_context(tc.tile_pool(name="const", bufs=1))
        res = ctx.enter_context(tc.tile_pool(name="res", bufs=1))
        wpool = ctx.enter_context(tc.tile_pool(name="wpool", bufs=2))
        xs = ctx.enter_context(tc.tile_pool(name="xs", bufs=2))
        stg = ctx.enter_context(tc.tile_pool(name="stg", bufs=2))
        p8p = ctx.enter_context(tc.tile_pool(name="p8p", bufs=2))
        spool = ctx.enter_context(tc.tile_pool(name="spool", bufs=1))
        lnp = ctx.enter_context(tc.tile_pool(name="lnp", bufs=2))
        rowp = ctx.enter_context(tc.tile_pool(name="rowp", bufs=1))
        ps_sc = ctx.enter_context(tc.tile_pool(name="ps_sc", bufs=2, space="PSUM"))
        ps_ov = ctx.enter_context(tc.tile_pool(name="ps_ov", bufs=1, space="PSUM"))
        ps_pr = ctx.enter_context(tc.tile_pool(name="ps_pr", bufs=2, space="PSUM"))
        ps_ln = ctx.enter_context(tc.tile_pool(name="ps_ln", bufs=1, space="PSUM"))

        ident = const.tile([P, P], bf16)
        make_identity(nc, ident[:])
        eps_t = const.tile([P, 1], f32, name="eps")
        nc.gpsimd.memset(eps_t[:], EPS)

        # ---------------- resident activation tiles ----------------
        # q8: zero-interleaved [Q0 | 0 | Q1 | 0] for the scores DoubleRow trick
        q8_t = res.tile([P, DT, 2 * TQ], fp8, name="q8_t", tag="qt8")
        q8_f = res.tile([P, DT, 2 * TQ], fp8, name="q8_f", tag="qf8")
        # K^T resident: [p(feat of head-pair), hp, kt(+pad), 128 keys]
        k8_f = res.tile([P, DT, KT + 1, P], fp8, name="k8_f", tag="k8f")
        k8_t = res.tile([P, DT, KT + 1, P], fp8, name="k8_t", tag="k8t")
        # V' resident token-major: [p(token), head, kt, VP]
        v8_f = res.tile([P, NH, KT, VP], fp8, name="v8_f", tag="v8f")
        v8_t = res.tile([P, NH, KT, VP], fp8, name="v8_t", tag="v8t")
        # normalized attention outputs O^T: [p(feat), dt, tq]
        o8_t = res.tile([P, DT, TQ], fp8, name="o8_t", tag="o8t")

        # one-time memsets: q8 zero slots, k8 pad tiles, v8 ones+pad cols
        for q8 in (q8_t, q8_f):
            nc.gpsimd.memset(q8[:, :, 512:1024], 0.0)
            nc.gpsimd.memset(q8[:, :, 1536:2048], 0.0)
        for k8 in (k8_f, k8_t):
            nc.gpsimd.memset(k8[:, :, KT, :], 0.0)
        for v8 in (v8_f, v8_t):
            nc.gpsimd.memset(v8[:, :, :, HD: HD + 1], 1.0)
            nc.gpsimd.memset(v8[:, :, :, HD + 1:], 0.0)

        bias_col = {}

        def load_bias_cols():
            for n in ["qt", "kf", "qf", "kt", "ot", "of", "f1", "f2"]:
                t = const.tile([P, DT], f32, name=f"bias_{n}")
                nc.sync.dma_start(t[:], b_d[n][:, :])
                bias_col[n] = t

        def row_bcast(dram_t, tag):
            """[1, D] bf16 dram row -> [128, D] bf16 broadcast tile."""
            r = rowp.tile([1, D], bf16, tag="row")
            nc.sync.dma_start(r[:], dram_t)
            b = rowp.tile([P, D], bf16, tag=tag)
            nc.gpsimd.partition_broadcast(b[:], r[:])
            return b

        def load_weight(name, kts=None):
            dram_t = w_d[name]
            if kts is None:
                kts = (0, dram_t.shape[1])
            nkt = kts[1] - kts[0]
            t = wpool.tile([P, nkt, D], fp8, tag="w")
            # two DMAs so the first pair-matmuls can start early
            h = nkt // 2
            nc.sync.dma_start(t[:, 0:h, :], dram_t[:, kts[0]: kts[0] + h, :])
            nc.sync.dma_start(t[:, h:nkt, :], dram_t[:, kts[0] + h: kts[1], :])
            return t

        def weight_loader(name, kts=None):
            """Lazy load: the DMA is emitted at first get() so the SP queue
            position sits after the consumers of the evicted slot."""
            state = {}

            def get(name=name, kts=kts):
                if "t" not in state:
                    state["t"] = load_weight(name, kts)
                return state["t"]

            return get

        # ------------------------------------------------------------------
        # unit builders
        # ------------------------------------------------------------------
        def x_block_loader(x_dram, bi):
            blk = {}

            def get():
                if "xb" not in blk:
                    xb = xs.tile([P, DT, 512], fp8, tag="xs")
                    nc.sync.dma_start(xb[:], x_dram[bi])
                    blk["xb"] = xb
                return blk["xb"]

            return get

        def featmaj_units(get_w, get_rhs, sink):
            """y^T[dout 128-tile, 512 tokens] via 4 DoubleRow matmuls."""
            units = []
            for dt in range(DT):

                def u(dt=dt):
                    w_sb = get_w()
                    nkt = w_sb.shape[1]
                    ps = ps_pr.tile([P, 512], f32, tag="prps")
                    rhs = get_rhs()
                    for i in range(nkt // 2):
                        nc.tensor.matmul(
                            ps[:],
                            w_sb[:, 2 * i: 2 * i + 2, dt * P: (dt + 1) * P],
                            rhs[:, 2 * i: 2 * i + 2, :],
                            start=(i == 0),
                            stop=(i == nkt // 2 - 1),
                            perf_mode=PM.DoubleRow,
                        )
                    sink(dt, ps[:])

                units.append(u)
            return units

        def sink_write(dest, ps, bname, dt, eng):
            """(psum/32 + bias) -> dest, on DVE or ACT."""
            if eng == "act":
                nc.scalar.activation(
                    dest, ps, AF.Identity,
                    bias=bias_col[bname][:, dt: dt + 1], scale=1.0 / WS,
                )
            else:
                nc.vector.tensor_scalar(
                    dest, ps, 1.0 / WS,
                    bias_col[bname][:, dt: dt + 1],
                    op0=ALU.mult, op1=ALU.add,
                )

        def qk_sink(bname, dest_ap_fn, eng="dve"):
            def sink(dt, ps):
                e = eng if eng != "alt" else ("dve" if dt % 2 else "act")
                sink_write(dest_ap_fn(dt), ps, bname, dt, e)
            return sink

        def qk_sink_k(k8, kt0, bname, eng="dve"):
            def sink(dt, ps):
                e = eng if eng != "alt" else ("dve" if dt % 2 else "act")
                sink_write(
                    k8[:, dt, kt0: kt0 + 4, :].rearrange("p a b -> p (a b)"),
                    ps, bname, dt, e)
            return sink

        def v_units(get_w, vb_bc, get_x, n0, v8):
            """token-major V' units for token block n0 (4 chunks x 2 halves)."""
            units = []
            for tci in range(4):
                for no in range(2):

                    def u(tci=tci, no=no):
                        w_sb = get_w()
                        ps = ps_pr.tile([P, 512], f32, tag="prps")
                        xb = get_x()
                        for i in range(DT // 2):
                            nc.tensor.matmul(
                                ps[:],
                                xb[:, 2 * i: 2 * i + 2, tci * P: (tci + 1) * P],
                                w_sb[:, 2 * i: 2 * i + 2,
                                     no * 512: (no + 1) * 512],
                                start=(i == 0),
                                stop=(i == DT // 2 - 1),
                                perf_mode=PM.DoubleRow,
                            )
                        kt_idx = (n0 + tci * P) // P
                        nc.gpsimd.scalar_tensor_tensor(
                            v8[:, no * 8: (no + 1) * 8, kt_idx, 0:HD],
                            ps.rearrange("p (h e) -> p h e", h=8), 1.0 / WS,
                            vb_bc[:, no * 512: (no + 1) * 512]
                            .rearrange("p (h e) -> p h e", h=8),
                            op0=ALU.mult, op1=ALU.add,
                        )

                    units.append(u)
            return units

        def attention_units(q8, k8, v8, o_sink):
            """Per (head, qt-512): 8 groups of 2 key-tiles + a norm tail.

            Returns a flat closure list at group granularity for fine
            interleaving. o_sink(h, qt, o_ps) consumes the accumulated
            [VP, 512] psum (rows 0:64 raw O, row 64 softmax denom).
            """
            by_qt = []
            for qt in range(NQ):
                units = []
                by_qt.append(units)
                for h in range(NH):
                    hp, s = h // 2, h % 2
                    state = {}

                    def grp(g, h=h, qt=qt, hp=hp, s=s, state=state):
                        def u():
                            sc = ps_sc.tile([P, 2, 512], f32, tag="scps")
                            for j in range(2):
                                kt = 2 * g + j
                                nc.tensor.matmul(
                                    sc[:, j, :],
                                    k8[s * HD: (s + 1) * HD, hp, kt: kt + 2, :],
                                    q8[s * HD: (s + 1) * HD, hp,
                                       qt * TQ: qt * TQ + TQ]
                                    .rearrange("p (a b) -> p a b", a=2),
                                    start=True, stop=True,
                                    perf_mode=PM.DoubleRow,
                                )
                            p8 = p8p.tile([P, 2, 512], fp8, tag="p8")
                            nc.scalar.activation(p8[:], sc[:], AF.Exp,
                                                 scale=1.0 / 8.0)
                            if g == 0:
                                ovps = ps_ov.tile([P, 512], f32, tag="ovps",
                                                  name="ovps")
                                state["ops"] = ovps
                            nc.tensor.matmul(
                                state["ops"][0:VP, :],
                                v8[:, h, 2 * g: 2 * g + 2, :],
                                p8[:],
                                start=(g == 0), stop=(g == 7),
                                perf_mode=PM.DoubleRow,
                            )
                        return u

                    for g in range(8):
                        units.append(grp(g))

                    def norm(h=h, qt=qt, state=state):
                        o_sink(h, qt, state["ops"])

                    units.append(norm)
            return by_qt

        def make_o_sink(o8):
            def o_sink(h, qt, ops):
                s = h % 2
                inv = spool.tile([1, 512], f32, tag="inv")
                nc.vector.reciprocal(inv[:], ops[HD: HD + 1, :])
                bc = spool.tile([HD, 512], f32, tag="bc")
                nc.gpsimd.partition_broadcast(bc[:], inv[:])
                if s == 0:
                    nc.vector.tensor_mul(
                        o8[0:HD, h // 2, qt * 512: (qt + 1) * 512],
                        ops[0:HD, :], bc[:])
                else:
                    tmp = stg.tile([HD, 512], fp8, tag="o8tmp")
                    nc.vector.tensor_mul(tmp[:], ops[0:HD, :], bc[:])
                    nc.sync.dma_start(
                        o8[HD:P, h // 2, qt * 512: (qt + 1) * 512], tmp[:])
            return o_sink

        def oproj_units(w_sb, bname, o8, attnT, eng="dve"):
            units = []
            for n0 in range(0, TQ, 512):
                for dt in range(DT):

                    def u(dt=dt, n0=n0):
                        ps = ps_pr.tile([P, 512], f32, tag="prps")
                        for i in range(DT // 2):
                            nc.tensor.matmul(
                                ps[:],
                                w_sb[:, 2 * i: 2 * i + 2, dt * P: (dt + 1) * P],
                                o8[:, 2 * i: 2 * i + 2, n0: n0 + 512],
                                start=(i == 0),
                                stop=(i == DT // 2 - 1),
                                perf_mode=PM.DoubleRow,
                            )
                        e = eng if eng != "blk" else ("dve" if n0 == 0 else "act")
                        sink_write(attnT[:, dt, n0: n0 + 512], ps, bname, dt, e)

                    units.append(u)
            return units

        def ln_units(inT, resid_dram, get_wb, out8, out_dram=None):
            """Token-major LN, one unit per 128-token chunk."""
            units = []
            for qc in range(QC):

                def u(qc=qc):
                    w_bc, b_bc = get_wb()
                    tok = ps_ln.tile([P, D], bf16, tag="lntok")
                    for dt in range(DT):
                        nc.tensor.transpose(
                            tok[:, dt * P: (dt + 1) * P],
                            inT[:, dt, qc * P: (qc + 1) * P],
                            ident[:],
                        )
                    if resid_dram is not None:
                        s = lnp.tile([P, D], bf16, tag="lnB")
                        xq = lnp.tile([P, D], bf16, tag="lnA")
                        nc.sync.dma_start(
                            xq[:], resid_dram[qc * P: (qc + 1) * P, :]
                        )
                        nc.vector.tensor_add(s[:], xq[:], tok[:])
                    else:
                        s = tok
                    bns = spool.tile([P, 2, 6], f32, tag="bns")
                    nc.vector.bn_stats(bns[:, 0, :], s[:, 0:512])
                    nc.vector.bn_stats(bns[:, 1, :], s[:, 512:D])
                    mv = spool.tile([P, 2], f32, tag="mv")
                    nc.vector.bn_aggr(mv[:], bns[:])
                    std = spool.tile([P, 1], f32, tag="std")
                    nc.scalar.activation(std[:], mv[:, 1:2], AF.Sqrt,
                                         bias=eps_t[:])
                    rstd = spool.tile([P, 1], f32, tag="rstd")
                    nc.vector.reciprocal(rstd[:], std[:])
                    t1 = lnp.tile([P, D], bf16, tag="lnA")
                    nc.vector.scalar_tensor_tensor(
                        t1[:], s[:], mv[:, 0:1], w_bc[:],
                        op0=ALU.subtract, op1=ALU.mult,
                    )
                    if out_dram is not None:
                        o = lnp.tile([P, D], f32, tag="lnB")
                        nc.vector.scalar_tensor_tensor(
                            o[:], t1[:], rstd[:], b_bc[:],
                            op0=ALU.mult, op1=ALU.add,
                        )
                        nc.sync.dma_start(out_dram[qc * P: (qc + 1) * P, :], o[:])
                    else:
                        nrm = lnp.tile([P, D], bf16, tag="lnnrm")
                        nc.vector.scalar_tensor_tensor(
                            nrm[:], t1[:], rstd[:], b_bc[:],
                            op0=ALU.mult, op1=ALU.add,
                        )
                        ft = ps_ln.tile([P, D], bf16, tag="lntok")
                        for dt in range(DT):
                            nc.tensor.transpose(
                                ft[:, dt * P: (dt + 1) * P],
                                nrm[:, dt * P: (dt + 1) * P],
                                ident[:],
                            )
                        nc.vector.tensor_copy(
                            out8[:, :, qc * P: (qc + 1) * P],
                            ft.rearrange("p (dt c) -> p dt c", dt=DT),
                        )

                units.append(u)
            return units

        def run_interleaved(primary, filler):
            k = 0
            for i, u in enumerate(primary):
                u()
                want = (i + 1) * len(filler) // len(primary)
                while k < want:
                    filler[k]()
                    k += 1
            while k < len(filler):
                filler[k]()
                k += 1

        # ------------------------------------------------------------------
        # program
        # ------------------------------------------------------------------
        # Phase 1 (prefix kept minimal: attention-1 needs only Kf + Qt +
        # Vf-block0; remaining Vf blocks become attention fillers).
        loaders_f = [x_block_loader(xfT_d, bi) for bi in range(4)]
        loaders_f[0]()
        g_kf = weight_loader("kf")
        g_kf()
        load_bias_cols()
        g_qt = weight_loader("qt")
        for bi in range(4):
            for u in featmaj_units(
                g_kf, loaders_f[bi],
                qk_sink_k(k8_f, bi * 4, "kf", eng="alt"),
            ):
                u()
        for qt in range(NQ):
            get_x = x_block_loader(xtT_d, qt)
            for u in featmaj_units(
                g_qt, get_x,
                qk_sink("qt", lambda dt, qt=qt: q8_t[:, dt, qt * TQ:
                                                     qt * TQ + 512],
                        eng="alt"),
            ):
                u()
        g_vf = weight_loader("vf")
        vb_f = row_bcast(br_d["vf"][:, :], "vbc1")
        loaders_fv = [x_block_loader(xfT_d, bi) for bi in range(4)]
        for u in v_units(g_vf, vb_f, loaders_fv[0], 0, v8_f):
            u()

        # Phase 2: attention-1 || Vf(1-3)/Kt+Vt/Qf.  Weight loads are lazy
        # (the load lands in the SP stream at its consumer's position), and
        # Kt/Vt units interleave per block so they share one x load within
        # the xs pool window.
        g_kt = weight_loader("kt")
        g_vt = weight_loader("vt")
        g_qf = weight_loader("qf")
        vb_t = row_bcast(br_d["vt"][:, :], "vbc2")
        loaders_t = [x_block_loader(xtT_d, bi) for bi in range(4)]
        fillers = []
        for bi in range(1, 4):
            fillers += v_units(g_vf, vb_f, loaders_fv[bi], bi * 512, v8_f)
        fillers.insert(len(fillers) - 2, lambda: (g_kt(), None)[1])
        fillers.insert(len(fillers) - 1, lambda: (g_vt(), None)[1])
        for bi in range(4):
            fillers += featmaj_units(
                g_kt, loaders_t[bi], qk_sink_k(k8_t, bi * 4, "kt"))
            fillers += v_units(g_vt, vb_t, loaders_t[bi], bi * 512, v8_t)
        fillers.insert(len(fillers) - 2, lambda: (g_qf(), None)[1])
        for qt in range(NQ):
            get_x = x_block_loader(xfT_d, qt)
            fillers += featmaj_units(
                g_qf, get_x,
                qk_sink("qf", lambda dt, qt=qt: q8_f[:, dt, qt * TQ:
                                                     qt * TQ + 512]),
            )
        a1_qt0, a1_qt1 = attention_units(q8_t, k8_f, v8_f, make_o_sink(o8_t))
        nhalf = len(fillers) // 2
        run_interleaved(a1_qt0, fillers[:nhalf])
        run_interleaved(a1_qt1, fillers[nhalf:])

        # Phase 3: attention-2. qt0 half || O-proj(t)+LN(t); qt1 half ||
        # block-0 endgame (O-proj(f) blk0 + LN(f) 0-3).
        w_ot = load_weight("ot")
        attnT_t = res.tile([P, DT, TQ], bf16, name="attnT_t", tag="big")
        fused8_t = res.tile([P, DT, TQ], fp8, name="fused8_t", tag="qt8")
        o8_f = res.tile([P, DT, TQ], fp8, name="o8_f", tag="k8f")
        lnt_wb = row_bcast(ln_d["lnt_w"].rearrange("(a d) -> a d", a=1), "lnw1")
        lnt_bb = row_bcast(ln_d["lnt_b"].rearrange("(a d) -> a d", a=1), "lnb1")
        oprojA = oproj_units(w_ot, "ot", o8_t, attnT_t)
        lnA = ln_units(attnT_t, xtq_d, lambda: (lnt_wb, lnt_bb), fused8_t)
        fillers2 = oprojA[:8]
        for i in range(4):
            fillers2.append(oprojA[8 + 2 * i])
            fillers2.append(oprojA[9 + 2 * i])
            fillers2.append(lnA[i])
        fillers2 += lnA[4:]

        w_of = load_weight("of")
        attnT_f = res.tile([P, DT, TQ], bf16, name="attnT_f", tag="big")
        fused8_f = res.tile([P, DT, TQ], fp8, name="fused8_f", tag="ff8")
        lnf_wb = row_bcast(ln_d["lnf_w"].rearrange("(a d) -> a d", a=1), "lnw2")
        lnf_bb = row_bcast(ln_d["lnf_b"].rearrange("(a d) -> a d", a=1), "lnb2")
        oprojB = oproj_units(w_of, "of", o8_f, attnT_f, eng="blk")
        lnB = ln_units(attnT_f, xfq_d, lambda: (lnf_wb, lnf_bb), fused8_f)

        a2_qt0, a2_qt1 = attention_units(q8_f, k8_t, v8_t, make_o_sink(o8_f))
        run_interleaved(a2_qt0, fillers2)
        fillers3 = oprojB[:8] + lnB[:4]
        run_interleaved(a2_qt1, fillers3)

        # Phase 4 tail: O-proj(f) blk1, LN(f) 4-7, fus1, fus2, LN(fus)
        g_f1a = weight_loader("f1", kts=(0, DT))
        g_f1b = weight_loader("f1", kts=(DT, 2 * DT))
        h8 = res.tile([P, DT, TQ], fp8, name="h8", tag="h8t")

        def fus1_units(n0):
            units = []
            for dt in range(DT):

                def u(dt=dt, n0=n0):
                    w_f1a = g_f1a()
                    w_f1b = g_f1b()
                    ps = ps_pr.tile([P, 512], f32, tag="prps")
                    for i in range(DT // 2):
                        nc.tensor.matmul(
                            ps[:],
                            w_f1a[:, 2 * i: 2 * i + 2, dt * P: (dt + 1) * P],
                            fused8_t[:, 2 * i: 2 * i + 2, n0: n0 + 512],
                            start=(i == 0), stop=False,
                            perf_mode=PM.DoubleRow,
                        )
                    for i in range(DT // 2):
                        nc.tensor.matmul(
                            ps[:],
                            w_f1b[:, 2 * i: 2 * i + 2, dt * P: (dt + 1) * P],
                            fused8_f[:, 2 * i: 2 * i + 2, n0: n0 + 512],
                            start=False, stop=(i == DT // 2 - 1),
                            perf_mode=PM.DoubleRow,
                        )
                    nc.scalar.activation(
                        h8[:, dt, n0: n0 + 512], ps[:], AF.Gelu,
                        bias=bias_col["f1"][:, dt: dt + 1], scale=1.0 / WS,
                    )

                units.append(u)
            return units

        g_f2 = weight_loader("f2")
        o2T = res.tile([P, DT, TQ], bf16, name="o2T", tag="v8f")
        lnu_state = {}

        def get_lnu():
            # lazy: reuses the lnt broadcast tags once lnA is finished
            if "w" not in lnu_state:
                lnu_state["w"] = row_bcast(
                    ln_d["lnu_w"].rearrange("(a d) -> a d", a=1), "lnw1")
                lnu_state["b"] = row_bcast(
                    ln_d["lnu_b"].rearrange("(a d) -> a d", a=1), "lnb1")
            return lnu_state["w"], lnu_state["b"]

        lnU = ln_units(o2T, None, get_lnu, None, out_dram=out_d)

        def fus2_units(n0, eng):
            return featmaj_units(
                g_f2, lambda n0=n0: h8[:, :, n0: n0 + 512],
                qk_sink("f2", lambda dt, n0=n0: o2T[:, dt, n0: n0 + 512],
                        eng=eng),
            )

        run_interleaved(oprojB[8:],
                        [lnB[4], lambda: (g_f1a(), None)[1],
                         lnB[5], lambda: (g_f1b(), None)[1]])
        run_interleaved(fus1_units(0),
                        [lnB[6], lambda: (g_f2(), None)[1], lnB[7]])
        for u in fus1_units(512):
            u()
        for u in fus2_units(0, "dve"):
            u()
        run_interleaved(fus2_units(512, "act"), lnU[:4])
        for u in lnU[4:]:
            u()

    nc.compile()
    return nc


# ---------------------------------------------------------------------------
# host side
# ---------------------------------------------------------------------------
_CACHE = {}


def _get_nc():
    if "nc" not in _CACHE:
        _CACHE["nc"] = _build_nc()
    return _CACHE["nc"]


def _make_in_maps(inputs):
    import ml_dtypes

    F8 = ml_dtypes.float8_e4m3

    def q8(a):
        return np.clip(a, -240.0, 240.0).astype(F8)

    def wshuf(w):
        # [din, dout] -> partition-major [128, din/128, dout], x32, fp8
        w = np.asarray(w, np.float32) * WS
        nkt = w.shape[0] // P
        return q8(np.ascontiguousarray(
            w.reshape(nkt, P, w.shape[1]).transpose(1, 0, 2)))

    def xshuf(xT):
        # [D, T] -> [T/512 blocks, 128, DT, 512], fp8
        return q8(np.ascontiguousarray(
            xT.reshape(DT, P, T // 512, 512).transpose(2, 1, 0, 3)))

    t = np.asarray(inputs["temporal_tokens"], np.float32)
    f = np.asarray(inputs["feature_tokens"], np.float32)

    def bshuf(b):
        return np.ascontiguousarray(
            np.asarray(b, np.float32).reshape(DT, P).T)

    shared = {}
    for n in _WNAMES:
        shared[f"w_{n}"] = wshuf(inputs[f"{n}_w"])
        shared[f"b_{n}"] = bshuf(inputs[f"{n}_b"])
    shared["w_f1"] = wshuf(inputs["fus1_w"])
    shared["b_f1"] = bshuf(inputs["fus1_b"])
    shared["w_f2"] = wshuf(inputs["fus2_w"])
    shared["b_f2"] = bshuf(inputs["fus2_b"])
    bf16 = ml_dtypes.bfloat16
    for n in ["vf", "vt"]:
        shared[f"br_{n}"] = np.ascontiguousarray(
            np.asarray(inputs[f"{n}_b"], np.float32).reshape(1, D)
        ).astype(bf16)
    for src, dst in [
        ("ln_t_w", "lnt_w"), ("ln_t_b", "lnt_b"),
        ("ln_f_w", "lnf_w"), ("ln_f_b", "lnf_b"),
        ("ln_fus_w", "lnu_w"), ("ln_fus_b", "lnu_b"),
    ]:
        shared[dst] = np.ascontiguousarray(inputs[src]).astype(bf16)

    in_maps = []
    for c in range(8):
        b, half = divmod(c, 2)
        r0 = half * TQ
        xt = t[b]
        xf = f[b]
        # query rows first, remaining rows after (K/V order is irrelevant)
        perm = np.concatenate([np.arange(r0, T), np.arange(0, r0)])
        m = dict(shared)
        m["xtT"] = xshuf(xt[perm].T)
        m["xfT"] = xshuf(xf[perm].T)
        m["xtq"] = np.ascontiguousarray(xt[r0: r0 + TQ]).astype(bf16)
        m["xfq"] = np.ascontiguousarray(xf[r0: r0 + TQ]).astype(bf16)
        in_maps.append(m)
    return in_maps


def kernel(**inputs):
    try:
        import jax

        jax.config.update("jax_compilation_cache_dir", "/tmp/jaxcache")
        jax.config.update("jax_persistent_cache_min_entry_size_bytes", -1)
        jax.config.update("jax_persistent_cache_min_compile_time_secs", 0.0)
    except Exception:
        pass
    from concourse.bass_utils import run_bass_kernel_spmd

    nc = _get_nc()
    in_maps = _make_in_maps(inputs)
    res = run_bass_kernel_spmd(nc, in_maps, list(range(8)))
    B = 4
    out = np.empty((B, T, D), np.float32)
    for c in range(8):
        b, half = divmod(c, 2)
        out[b, half * TQ: (half + 1) * TQ] = res.results[c]["out"]
    return out


# revision 53
# speedup vs baseline: 1.2744x; 1.0141x over previous
"""CoAttentionFusion Trainium2 kernel (8 NeuronCores, SPMD, no collectives).

Sharding: core c = (batch b = c//2, query-half h = c%2). Each core computes
the full module for its 1024 query rows of batch b; K/V projections over the
full T=2048 are recomputed by both cores of a batch pair.

v2: fp8e4m3 + DoubleRow matmuls everywhere.
  - weights pre-scaled x32 into e4m3's normal range, de-scaled (x1/32) for
    free in the bias sinks; activations quantized to e4m3 at each sink.
  - projections: contraction 1024 = 4 DoubleRow pairs (2 k-tiles/matmul).
  - scores: per-head contraction 64 with a zero-interleaved Q layout
    ([Q0 | 0 | Q1 | 0]) so the DoubleRow pair's second subtile multiplies
    zeros: 128 keys x 512 queries per 256-cycle matmul.
  - PV: V' stored [token, kt, 80] (64 feats + ones col + pad to the 16B
    dual-fp8 stride rule); 2 key-tiles per DoubleRow matmul.
  - K/V/O/Q all SBUF-resident in fp8 (no DRAM scratch roundtrips).
  - ACT runs only exp (softmax) + gelu + LN-sqrt; all bias sinks on DVE,
    V sinks and broadcasts on GPSIMD. ACT exp is the critical path.
  - attention emitted at 2-keytile group granularity; projection/LN units
    interleave as PE filler to keep the PE warm under the ACT-bound phases.
"""

import numpy as np

P = 128
D = 1024
T = 2048
TQ = 1024
NH = 16
HD = 64
DT = D // P          # 8 feature tiles
KT = T // P          # 16 key-token tiles
QC = TQ // P         # 8 query-token chunks
NQ = TQ // 512       # 2 query free-dim tiles
VP = 80              # V' padded cols: 64 feats + 1 ones + 15 pad
WS = 32.0            # weight prescale
EPS = 1e-5

_WNAMES = ["qt", "kf", "vf", "qf", "kt", "vt", "ot", "of"]


def _build_nc():
    import concourse.bass as bass
    import concourse.tile as tile
    from concourse import bacc, mybir
    from concourse.masks import make_identity
    from contextlib import ExitStack

    f32 = mybir.dt.float32
    bf16 = mybir.dt.bfloat16
    fp8 = mybir.dt.float8e4
    AF = mybir.ActivationFunctionType
    ALU = mybir.AluOpType
    PM = mybir.MatmulPerfMode

    nc = bacc.Bacc("TRN2", target_bir_lowering=False, debug=False, num_devices=8)

    # ---------------- DRAM I/O ----------------
    # x pre-blocked/partition-major fp8: [block, p, dt, 512]
    xtT_d = nc.dram_tensor("xtT", [T // 512, P, DT, 512], fp8,
                           kind="ExternalInput")
    xfT_d = nc.dram_tensor("xfT", [T // 512, P, DT, 512], fp8,
                           kind="ExternalInput")
    xtq_d = nc.dram_tensor("xtq", [TQ, D], bf16, kind="ExternalInput")
    xfq_d = nc.dram_tensor("xfq", [TQ, D], bf16, kind="ExternalInput")
    # weights pre-shuffled partition-major fp8 (x32): [p, kt, dout]
    w_d = {}
    b_d = {}
    for n in _WNAMES:
        w_d[n] = nc.dram_tensor(f"w_{n}", [P, DT, D], fp8, kind="ExternalInput")
        b_d[n] = nc.dram_tensor(f"b_{n}", [P, DT], f32, kind="ExternalInput")
    w_d["f1"] = nc.dram_tensor("w_f1", [P, 2 * DT, D], fp8, kind="ExternalInput")
    b_d["f1"] = nc.dram_tensor("b_f1", [P, DT], f32, kind="ExternalInput")
    w_d["f2"] = nc.dram_tensor("w_f2", [P, DT, D], fp8, kind="ExternalInput")
    b_d["f2"] = nc.dram_tensor("b_f2", [P, DT], f32, kind="ExternalInput")
    br_d = {}
    for n in ["vf", "vt"]:
        br_d[n] = nc.dram_tensor(f"br_{n}", [1, D], bf16, kind="ExternalInput")
    ln_d = {}
    for n in ["lnt_w", "lnt_b", "lnf_w", "lnf_b", "lnu_w", "lnu_b"]:
        ln_d[n] = nc.dram_tensor(n, [D], bf16, kind="ExternalInput")
    out_d = nc.dram_tensor("out", [TQ, D], bf16, kind="ExternalOutput")

    with tile.TileContext(nc) as tc, ExitStack() as ctx:
        const = ctx.enter_context(tc.tile_pool(name="const", bufs=1))
        res = ctx.enter_context(tc.tile_pool(name="res", bufs=1))
        wpool = ctx.enter_context(tc.tile_pool(name="wpool", bufs=2))
        xs = ctx.enter_context(tc.tile_pool(name="xs", bufs=2))
        stg = ctx.enter_context(tc.tile_pool(name="stg", bufs=2))
        p8p = ctx.enter_context(tc.tile_pool(name="p8p", bufs=2))
        spool = ctx.enter_context(tc.tile_pool(name="spool", bufs=1))
        lnp = ctx.enter_context(tc.tile_pool(name="lnp", bufs=2))
        rowp = ctx.enter_context(tc.tile_pool(name="rowp", bufs=1))
        ps_sc = ctx.enter_context(tc.tile_pool(name="ps_sc", bufs=2, space="PSUM"))
        ps_ov = ctx.enter_context(tc.tile_pool(name="ps_ov", bufs=1, space="PSUM"))
        ps_pr = ctx.enter_context(tc.tile_pool(name="ps_pr", bufs=2, space="PSUM"))
        ps_ln = ctx.enter_context(tc.tile_pool(name="ps_ln", bufs=1, space="PSUM"))

        ident = const.tile([P, P], bf16)
        make_identity(nc, ident[:])
        eps_t = const.tile([P, 1], f32, name="eps")
        nc.gpsimd.memset(eps_t[:], EPS)

        # ---------------- resident activation tiles ----------------
        # q8: zero-interleaved [Q0 | 0 | Q1 | 0] for the scores DoubleRow trick
        q8_t = res.tile([P, DT, 2 * TQ], fp8, name="q8_t", tag="qt8")
        q8_f = res.tile([P, DT, 2 * TQ], fp8, name="q8_f", tag="qf8")
        # K^T resident: [p(feat of head-pair), hp, kt(+pad), 128 keys]
        k8_f = res.tile([P, DT, KT + 1, P], fp8, name="k8_f", tag="k8f")
        k8_t = res.tile([P, DT, KT + 1, P], fp8, name="k8_t", tag="k8t")
        # V' resident token-major: [p(token), head, kt, VP]
        v8_f = res.tile([P, NH, KT, VP], fp8, name="v8_f", tag="v8f")
        v8_t = res.tile([P, NH, KT, VP], fp8, name="v8_t", tag="v8t")
        # normalized attention outputs O^T: [p(feat), dt, tq]
        o8_t = res.tile([P, DT, TQ], fp8, name="o8_t", tag="o8t")

        # one-time memsets: q8 zero slots, k8 pad tiles, v8 ones+pad cols
        for q8 in (q8_t, q8_f):
            nc.gpsimd.memset(q8[:, :, 512:1024], 0.0)
            nc.gpsimd.memset(q8[:, :, 1536:2048], 0.0)
        for k8 in (k8_f, k8_t):
            nc.gpsimd.memset(k8[:, :, KT, :], 0.0)
        for v8 in (v8_f, v8_t):
            nc.gpsimd.memset(v8[:, :, :, HD: HD + 1], 1.0)
            nc.gpsimd.memset(v8[:, :, :, HD + 1:], 0.0)

        bias_col = {}

        def load_bias_cols():
            for n in ["qt", "kf", "qf", "kt", "ot", "of", "f1", "f2"]:
                t = const.tile([P, DT], f32, name=f"bias_{n}")
                nc.sync.dma_start(t[:], b_d[n][:, :])
                bias_col[n] = t

        def row_bcast(dram_t, tag):
            """[1, D] bf16 dram row -> [128, D] bf16 broadcast tile."""
            r = rowp.tile([1, D], bf16, tag="row")
            nc.sync.dma_start(r[:], dram_t)
            b = rowp.tile([P, D], bf16, tag=tag)
            nc.gpsimd.partition_broadcast(b[:], r[:])
            return b

        def load_weight(name, kts=None):
            dram_t = w_d[name]
            if kts is None:
                kts = (0, dram_t.shape[1])
            nkt = kts[1] - kts[0]
            t = wpool.tile([P, nkt, D], fp8, tag="w")
            # two DMAs so the first pair-matmuls can start early
            h = nkt // 2
            nc.sync.dma_start(t[:, 0:h, :], dram_t[:, kts[0]: kts[0] + h, :])
            nc.sync.dma_start(t[:, h:nkt, :], dram_t[:, kts[0] + h: kts[1], :])
            return t

        def weight_loader(name, kts=None):
            """Lazy load: the DMA is emitted at first get() so the SP queue
            position sits after the consumers of the evicted slot."""
            state = {}

            def get(name=name, kts=kts):
                if "t" not in state:
                    state["t"] = load_weight(name, kts)
                return state["t"]

            return get

        # ------------------------------------------------------------------
        # unit builders
        # ------------------------------------------------------------------
        def x_block_loader(x_dram, bi):
            blk = {}

            def get():
                if "xb" not in blk:
                    xb = xs.tile([P, DT, 512], fp8, tag="xs")
                    nc.sync.dma_start(xb[:], x_dram[bi])
                    blk["xb"] = xb
                return blk["xb"]

            return get

        def featmaj_units(get_w, get_rhs, sink):
            """y^T[dout 128-tile, 512 tokens] via 4 DoubleRow matmuls."""
            units = []
            for dt in range(DT):

                def u(dt=dt):
                    w_sb = get_w()
                    nkt = w_sb.shape[1]
                    ps = ps_pr.tile([P, 512], f32, tag="prps")
                    rhs = get_rhs()
                    for i in range(nkt // 2):
                        nc.tensor.matmul(
                            ps[:],
                            w_sb[:, 2 * i: 2 * i + 2, dt * P: (dt + 1) * P],
                            rhs[:, 2 * i: 2 * i + 2, :],
                            start=(i == 0),
                            stop=(i == nkt // 2 - 1),
                            perf_mode=PM.DoubleRow,
                        )
                    sink(dt, ps[:])

                units.append(u)
            return units

        def sink_write(dest, ps, bname, dt, eng):
            """(psum/32 + bias) -> dest, on DVE or ACT."""
            if eng == "act":
                nc.scalar.activation(
                    dest, ps, AF.Identity,
                    bias=bias_col[bname][:, dt: dt + 1], scale=1.0 / WS,
                )
            else:
                nc.vector.tensor_scalar(
                    dest, ps, 1.0 / WS,
                    bias_col[bname][:, dt: dt + 1],
                    op0=ALU.mult, op1=ALU.add,
                )

        def qk_sink(bname, dest_ap_fn, eng="dve"):
            def sink(dt, ps):
                e = eng if eng != "alt" else ("dve" if dt % 2 else "act")
                sink_write(dest_ap_fn(dt), ps, bname, dt, e)
            return sink

        def qk_sink_k(k8, kt0, bname, eng="dve"):
            def sink(dt, ps):
                e = eng if eng != "alt" else ("dve" if dt % 2 else "act")
                sink_write(
                    k8[:, dt, kt0: kt0 + 4, :].rearrange("p a b -> p (a b)"),
                    ps, bname, dt, e)
            return sink

        def v_units(get_w, vb_bc, get_x, n0, v8):
            """token-major V' units for token block n0 (4 chunks x 2 halves)."""
            units = []
            for tci in range(4):
                for no in range(2):

                    def u(tci=tci, no=no):
                        w_sb = get_w()
                        ps = ps_pr.tile([P, 512], f32, tag="prps")
                        xb = get_x()
                        for i in range(DT // 2):
                            nc.tensor.matmul(
                                ps[:],
                                xb[:, 2 * i: 2 * i + 2, tci * P: (tci + 1) * P],
                                w_sb[:, 2 * i: 2 * i + 2,
                                     no * 512: (no + 1) * 512],
                                start=(i == 0),
                                stop=(i == DT // 2 - 1),
                                perf_mode=PM.DoubleRow,
                            )
                        kt_idx = (n0 + tci * P) // P
                        nc.gpsimd.scalar_tensor_tensor(
                            v8[:, no * 8: (no + 1) * 8, kt_idx, 0:HD],
                            ps.rearrange("p (h e) -> p h e", h=8), 1.0 / WS,
                            vb_bc[:, no * 512: (no + 1) * 512]
                            .rearrange("p (h e) -> p h e", h=8),
                            op0=ALU.mult, op1=ALU.add,
                        )

                    units.append(u)
            return units

        def attention_units(q8, k8, v8, o_sink):
            """Per (head, qt-512): 8 groups of 2 key-tiles + a norm tail.

            Returns a flat closure list at group granularity for fine
            interleaving. o_sink(h, qt, o_ps) consumes the accumulated
            [VP, 512] psum (rows 0:64 raw O, row 64 softmax denom).
            """
            by_qt = []
            for qt in range(NQ):
                units = []
                by_qt.append(units)
                for h in range(NH):
                    hp, s = h // 2, h % 2
                    state = {}

                    def pv(g, h=h, state=state):
                        nc.tensor.matmul(
                            state["ops"][0:VP, :],
                            v8[:, h, 2 * g: 2 * g + 2, :],
                            state.pop(g)[:],
                            start=(g == 0), stop=(g == 7),
                            perf_mode=PM.DoubleRow,
                        )

                    def grp(g, h=h, qt=qt, hp=hp, s=s, state=state, pv=pv):
                        # scores+exp for group g; PV for group g-1 (one-group
                        # skew so a stalled PV never delays the next exp)
                        def u():
                            sc = ps_sc.tile([P, 2, 512], f32, tag="scps")
                            for j in range(2):
                                kt = 2 * g + j
                                nc.tensor.matmul(
                                    sc[:, j, :],
                                    k8[s * HD: (s + 1) * HD, hp, kt: kt + 2, :],
                                    q8[s * HD: (s + 1) * HD, hp,
                                       qt * TQ: qt * TQ + TQ]
                                    .rearrange("p (a b) -> p a b", a=2),
                                    start=True, stop=True,
                                    perf_mode=PM.DoubleRow,
                                )
                            p8 = p8p.tile([P, 2, 512], fp8, tag="p8")
                            nc.scalar.activation(p8[:], sc[:], AF.Exp,
                                                 scale=1.0 / 8.0)
                            if g == 0:
                                ovps = ps_ov.tile([P, 512], f32, tag="ovps",
                                                  name="ovps")
                                state["ops"] = ovps
                            state[g] = p8
                            if g > 0:
                                pv(g - 1)
                        return u

                    for g in range(8):
                        units.append(grp(g))

                    def norm(h=h, qt=qt, state=state, pv=pv):
                        pv(7)
                        o_sink(h, qt, state["ops"])

                    units.append(norm)
            return by_qt

        def make_o_sink(o8):
            def o_sink(h, qt, ops):
                s = h % 2
                inv = spool.tile([1, 512], f32, tag="inv")
                nc.vector.reciprocal(inv[:], ops[HD: HD + 1, :])
                bc = spool.tile([HD, 512], f32, tag="bc")
                nc.gpsimd.partition_broadcast(bc[:], inv[:])
                if s == 0:
                    nc.vector.tensor_mul(
                        o8[0:HD, h // 2, qt * 512: (qt + 1) * 512],
                        ops[0:HD, :], bc[:])
                else:
                    tmp = stg.tile([HD, 512], fp8, tag="o8tmp")
                    nc.vector.tensor_mul(tmp[:], ops[0:HD, :], bc[:])
                    nc.sync.dma_start(
                        o8[HD:P, h // 2, qt * 512: (qt + 1) * 512], tmp[:])
            return o_sink

        def oproj_units(w_sb, bname, o8, attnT, eng="dve"):
            units = []
            for n0 in range(0, TQ, 512):
                for dt in range(DT):

                    def u(dt=dt, n0=n0):
                        ps = ps_pr.tile([P, 512], f32, tag="prps")
                        for i in range(DT // 2):
                            nc.tensor.matmul(
                                ps[:],
                                w_sb[:, 2 * i: 2 * i + 2, dt * P: (dt + 1) * P],
                                o8[:, 2 * i: 2 * i + 2, n0: n0 + 512],
                                start=(i == 0),
                                stop=(i == DT // 2 - 1),
                                perf_mode=PM.DoubleRow,
                            )
                        e = eng if eng != "blk" else ("dve" if n0 == 0 else "act")
                        sink_write(attnT[:, dt, n0: n0 + 512], ps, bname, dt, e)

                    units.append(u)
            return units

        def ln_units(inT, resid_dram, get_wb, out8, out_dram=None,
                     tail_from=QC):
            """Token-major LN, one unit per 128-token chunk. Chunks at
            qc >= tail_from alternate their PSUM tile with the (then idle)
            scores pool so consecutive chunks pipeline."""
            units = []
            for qc in range(QC):

                def u(qc=qc):
                    w_bc, b_bc = get_wb()

                    def mk_tok():
                        if qc >= tail_from and qc % 2:
                            return ps_sc.tile([P, D], bf16, tag="scps",
                                              name="lntok")
                        return ps_ln.tile([P, D], bf16, tag="lntok",
                                          name="lntok")

                    tok = mk_tok()
                    for dt in range(DT):
                        nc.tensor.transpose(
                            tok[:, dt * P: (dt + 1) * P],
                            inT[:, dt, qc * P: (qc + 1) * P],
                            ident[:],
                        )
                    if resid_dram is not None:
                        s = lnp.tile([P, D], bf16, tag="lnB")
                        xq = lnp.tile([P, D], bf16, tag="lnA")
                        nc.sync.dma_start(
                            xq[:], resid_dram[qc * P: (qc + 1) * P, :]
                        )
                        nc.vector.tensor_add(s[:], xq[:], tok[:])
                    else:
                        s = tok
                    bns = spool.tile([P, 2, 6], f32, tag="bns")
                    nc.vector.bn_stats(bns[:, 0, :], s[:, 0:512])
                    nc.vector.bn_stats(bns[:, 1, :], s[:, 512:D])
                    mv = spool.tile([P, 2], f32, tag="mv")
                    nc.vector.bn_aggr(mv[:], bns[:])
                    std = spool.tile([P, 1], f32, tag="std")
                    nc.scalar.activation(std[:], mv[:, 1:2], AF.Sqrt,
                                         bias=eps_t[:])
                    rstd = spool.tile([P, 1], f32, tag="rstd")
                    nc.vector.reciprocal(rstd[:], std[:])
                    t1 = lnp.tile([P, D], bf16, tag="lnA")
                    nc.vector.scalar_tensor_tensor(
                        t1[:], s[:], mv[:, 0:1], w_bc[:],
                        op0=ALU.subtract, op1=ALU.mult,
                    )
                    if out_dram is not None:
                        o = lnp.tile([P, D], bf16, tag="lnB")
                        nc.vector.scalar_tensor_tensor(
                            o[:], t1[:], rstd[:], b_bc[:],
                            op0=ALU.mult, op1=ALU.add,
                        )
                        nc.sync.dma_start(out_dram[qc * P: (qc + 1) * P, :], o[:])
                    else:
                        nrm = lnp.tile([P, D], bf16, tag="lnnrm")
                        nc.vector.scalar_tensor_tensor(
                            nrm[:], t1[:], rstd[:], b_bc[:],
                            op0=ALU.mult, op1=ALU.add,
                        )
                        ft = mk_tok()
                        for dt in range(DT):
                            nc.tensor.transpose(
                                ft[:, dt * P: (dt + 1) * P],
                                nrm[:, dt * P: (dt + 1) * P],
                                ident[:],
                            )
                        nc.vector.tensor_copy(
                            out8[:, :, qc * P: (qc + 1) * P],
                            ft.rearrange("p (dt c) -> p dt c", dt=DT),
                        )

                units.append(u)
            return units

        def run_interleaved(primary, filler):
            k = 0
            for i, u in enumerate(primary):
                u()
                want = (i + 1) * len(filler) // len(primary)
                while k < want:
                    filler[k]()
                    k += 1
            while k < len(filler):
                filler[k]()
                k += 1

        # ------------------------------------------------------------------
        # program
        # ------------------------------------------------------------------
        # Phase 1 (prefix kept minimal: attention-1 needs only Kf + Qt +
        # Vf-block0; remaining Vf blocks become attention fillers).
        loaders_f = [x_block_loader(xfT_d, bi) for bi in range(4)]
        loaders_f[0]()
        g_kf = weight_loader("kf")
        g_kf()
        load_bias_cols()
        g_qt = weight_loader("qt")
        for bi in range(4):
            for u in featmaj_units(
                g_kf, loaders_f[bi],
                qk_sink_k(k8_f, bi * 4, "kf", eng="alt"),
            ):
                u()
        for qt in range(NQ):
            get_x = x_block_loader(xtT_d, qt)
            for u in featmaj_units(
                g_qt, get_x,
                qk_sink("qt", lambda dt, qt=qt: q8_t[:, dt, qt * TQ:
                                                     qt * TQ + 512],
                        eng="alt"),
            ):
                u()
        g_vf = weight_loader("vf")
        vb_f = row_bcast(br_d["vf"][:, :], "vbc1")
        loaders_fv = [x_block_loader(xfT_d, bi) for bi in range(4)]
        for u in v_units(g_vf, vb_f, loaders_fv[0], 0, v8_f):
            u()

        # Phase 2: attention-1 || Vf(1-3)/Kt+Vt/Qf.  Weight loads are lazy
        # (the load lands in the SP stream at its consumer's position), and
        # Kt/Vt units interleave per block so they share one x load within
        # the xs pool window.
        g_kt = weight_loader("kt")
        g_vt = weight_loader("vt")
        g_qf = weight_loader("qf")
        vb_t = row_bcast(br_d["vt"][:, :], "vbc2")
        loaders_t = [x_block_loader(xtT_d, bi) for bi in range(4)]
        fillers = []
        for bi in range(1, 4):
            fillers += v_units(g_vf, vb_f, loaders_fv[bi], bi * 512, v8_f)
        fillers.insert(len(fillers) - 2, lambda: (g_kt(), None)[1])
        fillers.insert(len(fillers) - 1, lambda: (g_vt(), None)[1])
        for bi in range(4):
            fillers += featmaj_units(
                g_kt, loaders_t[bi], qk_sink_k(k8_t, bi * 4, "kt"))
            fillers += v_units(g_vt, vb_t, loaders_t[bi], bi * 512, v8_t)
        fillers.insert(len(fillers) - 2, lambda: (g_qf(), None)[1])
        for qt in range(NQ):
            get_x = x_block_loader(xfT_d, qt)
            fillers += featmaj_units(
                g_qf, get_x,
                qk_sink("qf", lambda dt, qt=qt: q8_f[:, dt, qt * TQ:
                                                     qt * TQ + 512]),
            )
        a1_qt0, a1_qt1 = attention_units(q8_t, k8_f, v8_f, make_o_sink(o8_t))
        nhalf = len(fillers) // 2
        run_interleaved(a1_qt0, fillers[:nhalf])
        run_interleaved(a1_qt1, fillers[nhalf:])

        # Phase 3: attention-2. qt0 half || O-proj(t)+LN(t); qt1 half ||
        # block-0 endgame (O-proj(f) blk0 + LN(f) 0-3).
        w_ot = load_weight("ot")
        attnT_t = res.tile([P, DT, TQ], bf16, name="attnT_t", tag="big")
        fused8_t = res.tile([P, DT, TQ], fp8, name="fused8_t", tag="qt8")
        o8_f = res.tile([P, DT, TQ], fp8, name="o8_f", tag="k8f")
        lnt_wb = row_bcast(ln_d["lnt_w"].rearrange("(a d) -> a d", a=1), "lnw1")
        lnt_bb = row_bcast(ln_d["lnt_b"].rearrange("(a d) -> a d", a=1), "lnb1")
        oprojA = oproj_units(w_ot, "ot", o8_t, attnT_t)
        lnA = ln_units(attnT_t, xtq_d, lambda: (lnt_wb, lnt_bb), fused8_t)
        fillers2 = oprojA[:8]
        for i in range(4):
            fillers2.append(oprojA[8 + 2 * i])
            fillers2.append(oprojA[9 + 2 * i])
            fillers2.append(lnA[i])
        fillers2 += lnA[4:]

        w_of = load_weight("of")
        attnT_f = res.tile([P, DT, TQ], bf16, name="attnT_f", tag="big")
        fused8_f = res.tile([P, DT, TQ], fp8, name="fused8_f", tag="ff8")
        lnf_wb = row_bcast(ln_d["lnf_w"].rearrange("(a d) -> a d", a=1), "lnw2")
        lnf_bb = row_bcast(ln_d["lnf_b"].rearrange("(a d) -> a d", a=1), "lnb2")
        oprojB = oproj_units(w_of, "of", o8_f, attnT_f, eng="blk")
        lnB = ln_units(attnT_f, xfq_d, lambda: (lnf_wb, lnf_bb), fused8_f,
                       tail_from=4)

        a2_qt0, a2_qt1 = attention_units(q8_f, k8_t, v8_t, make_o_sink(o8_f))
        run_interleaved(a2_qt0, fillers2)
        fillers3 = oprojB[:8] + lnB[:4]
        run_interleaved(a2_qt1, fillers3)

        # Phase 4 tail: O-proj(f) blk1, LN(f) 4-7, fus1, fus2, LN(fus)
        g_f1a = weight_loader("f1", kts=(0, DT))
        g_f1b = weight_loader("f1", kts=(DT, 2 * DT))
        h8 = res.tile([P, DT, TQ], fp8, name="h8", tag="h8t")

        def fus1_units(n0):
            units = []
            for dt in range(DT):

                def u(dt=dt, n0=n0):
                    w_f1a = g_f1a()
                    w_f1b = g_f1b()
                    ps = ps_pr.tile([P, 512], f32, tag="prps")
                    for i in range(DT // 2):
                        nc.tensor.matmul(
                            ps[:],
                            w_f1a[:, 2 * i: 2 * i + 2, dt * P: (dt + 1) * P],
                            fused8_t[:, 2 * i: 2 * i + 2, n0: n0 + 512],
                            start=(i == 0), stop=False,
                            perf_mode=PM.DoubleRow,
                        )
                    for i in range(DT // 2):
                        nc.tensor.matmul(
                            ps[:],
                            w_f1b[:, 2 * i: 2 * i + 2, dt * P: (dt + 1) * P],
                            fused8_f[:, 2 * i: 2 * i + 2, n0: n0 + 512],
                            start=False, stop=(i == DT // 2 - 1),
                            perf_mode=PM.DoubleRow,
                        )
                    nc.scalar.activation(
                        h8[:, dt, n0: n0 + 512], ps[:], AF.Gelu,
                        bias=bias_col["f1"][:, dt: dt + 1], scale=1.0 / WS,
                    )

                units.append(u)
            return units

        g_f2 = weight_loader("f2")
        o2T = res.tile([P, DT, TQ], bf16, name="o2T", tag="v8f")
        lnu_state = {}

        def get_lnu():
            # lazy: reuses the lnt broadcast tags once lnA is finished
            if "w" not in lnu_state:
                lnu_state["w"] = row_bcast(
                    ln_d["lnu_w"].rearrange("(a d) -> a d", a=1), "lnw1")
                lnu_state["b"] = row_bcast(
                    ln_d["lnu_b"].rearrange("(a d) -> a d", a=1), "lnb1")
            return lnu_state["w"], lnu_state["b"]

        lnU = ln_units(o2T, None, get_lnu, None, out_dram=out_d, tail_from=0)

        def fus2_units(n0, eng):
            return featmaj_units(
                g_f2, lambda n0=n0: h8[:, :, n0: n0 + 512],
                qk_sink("f2", lambda dt, n0=n0: o2T[:, dt, n0: n0 + 512],
                        eng=eng),
            )

        run_interleaved(oprojB[8:],
                        [lnB[4], lambda: (g_f1a(), None)[1],
                         lnB[5], lambda: (g_f1b(), None)[1]])
        run_interleaved(fus1_units(0),
                        [lnB[6], lambda: (g_f2(), None)[1], lnB[7]])
        for u in fus1_units(512):
            u()
        for u in fus2_units(0, "dve"):
            u()
        run_interleaved(fus2_units(512, "act"), lnU[:4])
        for u in lnU[4:]:
            u()

    nc.compile()
    return nc


# ---------------------------------------------------------------------------
# host side
# ---------------------------------------------------------------------------
_CACHE = {}


def _get_nc():
    if "nc" not in _CACHE:
        _CACHE["nc"] = _build_nc()
    return _CACHE["nc"]


def _make_in_maps(inputs):
    import ml_dtypes

    F8 = ml_dtypes.float8_e4m3

    def q8(a):
        return np.clip(a, -240.0, 240.0).astype(F8)

    def wshuf(w):
        # [din, dout] -> partition-major [128, din/128, dout], x32, fp8
        w = np.asarray(w, np.float32) * WS
        nkt = w.shape[0] // P
        return q8(np.ascontiguousarray(
            w.reshape(nkt, P, w.shape[1]).transpose(1, 0, 2)))

    def xshuf(xT):
        # [D, T] -> [T/512 blocks, 128, DT, 512], fp8
        return q8(np.ascontiguousarray(
            xT.reshape(DT, P, T // 512, 512).transpose(2, 1, 0, 3)))

    t = np.asarray(inputs["temporal_tokens"], np.float32)
    f = np.asarray(inputs["feature_tokens"], np.float32)

    def bshuf(b):
        return np.ascontiguousarray(
            np.asarray(b, np.float32).reshape(DT, P).T)

    shared = {}
    for n in _WNAMES:
        shared[f"w_{n}"] = wshuf(inputs[f"{n}_w"])
        shared[f"b_{n}"] = bshuf(inputs[f"{n}_b"])
    shared["w_f1"] = wshuf(inputs["fus1_w"])
    shared["b_f1"] = bshuf(inputs["fus1_b"])
    shared["w_f2"] = wshuf(inputs["fus2_w"])
    shared["b_f2"] = bshuf(inputs["fus2_b"])
    bf16 = ml_dtypes.bfloat16
    for n in ["vf", "vt"]:
        shared[f"br_{n}"] = np.ascontiguousarray(
            np.asarray(inputs[f"{n}_b"], np.float32).reshape(1, D)
        ).astype(bf16)
    for src, dst in [
        ("ln_t_w", "lnt_w"), ("ln_t_b", "lnt_b"),
        ("ln_f_w", "lnf_w"), ("ln_f_b", "lnf_b"),
        ("ln_fus_w", "lnu_w"), ("ln_fus_b", "lnu_b"),
    ]:
        shared[dst] = np.ascontiguousarray(inputs[src]).astype(bf16)

    in_maps = []
    for c in range(8):
        b, half = divmod(c, 2)
        r0 = half * TQ
        xt = t[b]
        xf = f[b]
        # query rows first, remaining rows after (K/V order is irrelevant)
        perm = np.concatenate([np.arange(r0, T), np.arange(0, r0)])
        m = dict(shared)
        m["xtT"] = xshuf(xt[perm].T)
        m["xfT"] = xshuf(xf[perm].T)
        m["xtq"] = np.ascontiguousarray(xt[r0: r0 + TQ]).astype(bf16)
        m["xfq"] = np.ascontiguousarray(xf[r0: r0 + TQ]).astype(bf16)
        in_maps.append(m)
    return in_maps


def kernel(**inputs):
    try:
        import jax

        jax.config.update("jax_compilation_cache_dir", "/tmp/jaxcache")
        jax.config.update("jax_persistent_cache_min_entry_size_bytes", -1)
        jax.config.update("jax_persistent_cache_min_compile_time_secs", 0.0)
    except Exception:
        pass
    from concourse.bass_utils import run_bass_kernel_spmd

    nc = _get_nc()
    in_maps = _make_in_maps(inputs)
    res = run_bass_kernel_spmd(nc, in_maps, list(range(8)))
    B = 4
    out = np.empty((B, T, D), np.float32)
    for c in range(8):
        b, half = divmod(c, 2)
        out[b, half * TQ: (half + 1) * TQ] = np.asarray(
            res.results[c]["out"]).astype(np.float32)
    return out


# revision 56
# speedup vs baseline: 1.3933x; 1.0933x over previous
"""CoAttentionFusion Trainium2 kernel (8 NeuronCores, SPMD, no collectives).

Sharding: core c = (batch b = c//2, query-half h = c%2). Each core computes
the full module for its 1024 query rows of batch b; K/V projections over the
full T=2048 are recomputed by both cores of a batch pair.

v2: fp8e4m3 + DoubleRow matmuls everywhere.
  - weights pre-scaled x32 into e4m3's normal range, de-scaled (x1/32) for
    free in the bias sinks; activations quantized to e4m3 at each sink.
  - projections: contraction 1024 = 4 DoubleRow pairs (2 k-tiles/matmul).
  - scores: per-head contraction 64 with a zero-interleaved Q layout
    ([Q0 | 0 | Q1 | 0]) so the DoubleRow pair's second subtile multiplies
    zeros: 128 keys x 512 queries per 256-cycle matmul.
  - PV: V' stored [token, kt, 80] (64 feats + ones col + pad to the 16B
    dual-fp8 stride rule); 2 key-tiles per DoubleRow matmul.
  - K/V/O/Q all SBUF-resident in fp8 (no DRAM scratch roundtrips).
  - ACT runs only exp (softmax) + gelu + LN-sqrt; all bias sinks on DVE,
    V sinks and broadcasts on GPSIMD. ACT exp is the critical path.
  - attention emitted at 2-keytile group granularity; projection/LN units
    interleave as PE filler to keep the PE warm under the ACT-bound phases.
"""

import numpy as np

P = 128
D = 1024
T = 2048
TQ = 1024
NH = 16
HD = 64
DT = D // P          # 8 feature tiles
KT = T // P          # 16 key-token tiles
QC = TQ // P         # 8 query-token chunks
NQ = TQ // 512       # 2 query free-dim tiles
VP = 80              # V' padded cols: 64 feats + 1 ones + 15 pad
WS = 32.0            # weight prescale
EPS = 1e-5

_WNAMES = ["qt", "kf", "vf", "qf", "kt", "vt", "ot", "of"]


def _build_nc():
    import concourse.bass as bass
    import concourse.tile as tile
    from concourse import bacc, mybir
    from concourse.masks import make_identity
    from contextlib import ExitStack

    f32 = mybir.dt.float32
    bf16 = mybir.dt.bfloat16
    fp8 = mybir.dt.float8e4
    AF = mybir.ActivationFunctionType
    ALU = mybir.AluOpType
    PM = mybir.MatmulPerfMode

    nc = bacc.Bacc("TRN2", target_bir_lowering=False, debug=False, num_devices=8)

    # ---------------- DRAM I/O ----------------
    # x pre-blocked/partition-major fp8: [block, p, dt, 512]
    xtT_d = nc.dram_tensor("xtT", [T // 512, P, DT, 512], fp8,
                           kind="ExternalInput")
    xfT_d = nc.dram_tensor("xfT", [T // 512, P, DT, 512], fp8,
                           kind="ExternalInput")
    xtq_d = nc.dram_tensor("xtq", [TQ, D], bf16, kind="ExternalInput")
    xfq_d = nc.dram_tensor("xfq", [TQ, D], bf16, kind="ExternalInput")
    # weights pre-shuffled partition-major fp8 (x32): [p, kt, dout]
    w_d = {}
    b_d = {}
    for n in _WNAMES:
        w_d[n] = nc.dram_tensor(f"w_{n}", [P, DT, D], fp8, kind="ExternalInput")
        b_d[n] = nc.dram_tensor(f"b_{n}", [P, DT], f32, kind="ExternalInput")
    w_d["f1"] = nc.dram_tensor("w_f1", [P, 2 * DT, D], fp8, kind="ExternalInput")
    b_d["f1"] = nc.dram_tensor("b_f1", [P, DT], f32, kind="ExternalInput")
    w_d["f2"] = nc.dram_tensor("w_f2", [P, DT, D], fp8, kind="ExternalInput")
    b_d["f2"] = nc.dram_tensor("b_f2", [P, DT], f32, kind="ExternalInput")
    br_d = {}
    for n in ["vf", "vt"]:
        br_d[n] = nc.dram_tensor(f"br_{n}", [1, D], bf16, kind="ExternalInput")
    ln_d = {}
    for n in ["lnt_w", "lnt_b", "lnf_w", "lnf_b", "lnu_w", "lnu_b"]:
        ln_d[n] = nc.dram_tensor(n, [D], bf16, kind="ExternalInput")
    out_d = nc.dram_tensor("out", [TQ, D], bf16, kind="ExternalOutput")

    with tile.TileContext(nc) as tc, ExitStack() as ctx:
        const = ctx.enter_context(tc.tile_pool(name="const", bufs=1))
        res = ctx.enter_context(tc.tile_pool(name="res", bufs=1))
        wpool = ctx.enter_context(tc.tile_pool(name="wpool", bufs=2))
        xs = ctx.enter_context(tc.tile_pool(name="xs", bufs=2))
        stg = ctx.enter_context(tc.tile_pool(name="stg", bufs=2))
        p8p = ctx.enter_context(tc.tile_pool(name="p8p", bufs=4))
        spool = ctx.enter_context(tc.tile_pool(name="spool", bufs=1))
        lnp = ctx.enter_context(tc.tile_pool(name="lnp", bufs=2))
        rowp = ctx.enter_context(tc.tile_pool(name="rowp", bufs=1))
        ps_sc = ctx.enter_context(tc.tile_pool(name="ps_sc", bufs=2, space="PSUM"))
        ps_ov = ctx.enter_context(tc.tile_pool(name="ps_ov", bufs=1, space="PSUM"))
        ps_pr = ctx.enter_context(tc.tile_pool(name="ps_pr", bufs=2, space="PSUM"))
        ps_ln = ctx.enter_context(tc.tile_pool(name="ps_ln", bufs=1, space="PSUM"))

        ident = const.tile([P, P], bf16)
        make_identity(nc, ident[:])
        eps_t = const.tile([P, 1], f32, name="eps")
        nc.gpsimd.memset(eps_t[:], EPS)

        # ---------------- resident activation tiles ----------------
        # q8: zero-interleaved [Q0 | 0 | Q1 | 0] for the scores DoubleRow trick
        q8_t = res.tile([P, DT, 2 * TQ], fp8, name="q8_t", tag="qt8")
        q8_f = res.tile([P, DT, 2 * TQ], fp8, name="q8_f", tag="qf8")
        # K^T resident: [p(feat of head-pair), hp, kt(+pad), 128 keys]
        k8_f = res.tile([P, DT, KT + 1, P], fp8, name="k8_f", tag="k8f")
        k8_t = res.tile([P, DT, KT + 1, P], fp8, name="k8_t", tag="k8t")
        # V' resident token-major: [p(token), head, kt, VP]
        v8_f = res.tile([P, NH, KT, VP], fp8, name="v8_f", tag="v8f")
        v8_t = res.tile([P, NH, KT, VP], fp8, name="v8_t", tag="v8t")
        # normalized attention outputs O^T: [p(feat), dt, tq]
        o8_t = res.tile([P, DT, TQ], fp8, name="o8_t", tag="o8t")

        # one-time memsets: q8 zero slots, k8 pad tiles, v8 ones+pad cols
        for q8 in (q8_t, q8_f):
            nc.gpsimd.memset(q8[:, :, 512:1024], 0.0)
            nc.gpsimd.memset(q8[:, :, 1536:2048], 0.0)
        for k8 in (k8_f, k8_t):
            nc.gpsimd.memset(k8[:, :, KT, :], 0.0)
        for v8 in (v8_f, v8_t):
            nc.gpsimd.memset(v8[:, :, :, HD: HD + 1], 1.0)
            nc.gpsimd.memset(v8[:, :, :, HD + 1:], 0.0)

        bias_col = {}

        def load_bias_cols():
            for n in ["qt", "kf", "qf", "kt", "ot", "of", "f1", "f2"]:
                t = const.tile([P, DT], f32, name=f"bias_{n}")
                nc.sync.dma_start(t[:], b_d[n][:, :])
                bias_col[n] = t

        def row_bcast(dram_t, tag):
            """[1, D] bf16 dram row -> [128, D] bf16 broadcast tile."""
            r = rowp.tile([1, D], bf16, tag="row")
            nc.sync.dma_start(r[:], dram_t)
            b = rowp.tile([P, D], bf16, tag=tag)
            nc.gpsimd.partition_broadcast(b[:], r[:])
            return b

        def load_weight(name, kts=None):
            dram_t = w_d[name]
            if kts is None:
                kts = (0, dram_t.shape[1])
            nkt = kts[1] - kts[0]
            t = wpool.tile([P, nkt, D], fp8, tag="w")
            # two DMAs so the first pair-matmuls can start early
            h = nkt // 2
            nc.sync.dma_start(t[:, 0:h, :], dram_t[:, kts[0]: kts[0] + h, :])
            nc.sync.dma_start(t[:, h:nkt, :], dram_t[:, kts[0] + h: kts[1], :])
            return t

        def weight_loader(name, kts=None):
            """Lazy load: the DMA is emitted at first get() so the SP queue
            position sits after the consumers of the evicted slot."""
            state = {}

            def get(name=name, kts=kts):
                if "t" not in state:
                    state["t"] = load_weight(name, kts)
                return state["t"]

            return get

        # ------------------------------------------------------------------
        # unit builders
        # ------------------------------------------------------------------
        def x_block_loader(x_dram, bi):
            blk = {}

            def get():
                if "xb" not in blk:
                    xb = xs.tile([P, DT, 512], fp8, tag="xs")
                    nc.sync.dma_start(xb[:], x_dram[bi])
                    blk["xb"] = xb
                return blk["xb"]

            return get

        def featmaj_units(get_w, get_rhs, sink):
            """y^T[dout 128-tile, 512 tokens] via 4 DoubleRow matmuls."""
            units = []
            for dt in range(DT):

                def u(dt=dt):
                    w_sb = get_w()
                    nkt = w_sb.shape[1]
                    ps = ps_pr.tile([P, 512], f32, tag="prps")
                    rhs = get_rhs()
                    for i in range(nkt // 2):
                        nc.tensor.matmul(
                            ps[:],
                            w_sb[:, 2 * i: 2 * i + 2, dt * P: (dt + 1) * P],
                            rhs[:, 2 * i: 2 * i + 2, :],
                            start=(i == 0),
                            stop=(i == nkt // 2 - 1),
                            perf_mode=PM.DoubleRow,
                        )
                    sink(dt, ps[:])

                units.append(u)
            return units

        def sink_write(dest, ps, bname, dt, eng):
            """(psum/32 + bias) -> dest, on DVE or ACT."""
            if eng == "act":
                nc.scalar.activation(
                    dest, ps, AF.Identity,
                    bias=bias_col[bname][:, dt: dt + 1], scale=1.0 / WS,
                )
            else:
                nc.vector.tensor_scalar(
                    dest, ps, 1.0 / WS,
                    bias_col[bname][:, dt: dt + 1],
                    op0=ALU.mult, op1=ALU.add,
                )

        def qk_sink(bname, dest_ap_fn, eng="dve"):
            def sink(dt, ps):
                e = eng if eng != "alt" else ("dve" if dt % 2 else "act")
                sink_write(dest_ap_fn(dt), ps, bname, dt, e)
            return sink

        def qk_sink_k(k8, kt0, bname, eng="dve"):
            def sink(dt, ps):
                e = eng if eng != "alt" else ("dve" if dt % 2 else "act")
                sink_write(
                    k8[:, dt, kt0: kt0 + 4, :].rearrange("p a b -> p (a b)"),
                    ps, bname, dt, e)
            return sink

        def v_units(get_w, vb_bc, get_x, n0, v8):
            """token-major V' units for token block n0 (4 chunks x 2 halves)."""
            units = []
            for tci in range(4):
                for no in range(2):

                    def u(tci=tci, no=no):
                        w_sb = get_w()
                        ps = ps_pr.tile([P, 512], f32, tag="prps")
                        xb = get_x()
                        for i in range(DT // 2):
                            nc.tensor.matmul(
                                ps[:],
                                xb[:, 2 * i: 2 * i + 2, tci * P: (tci + 1) * P],
                                w_sb[:, 2 * i: 2 * i + 2,
                                     no * 512: (no + 1) * 512],
                                start=(i == 0),
                                stop=(i == DT // 2 - 1),
                                perf_mode=PM.DoubleRow,
                            )
                        kt_idx = (n0 + tci * P) // P
                        nc.gpsimd.scalar_tensor_tensor(
                            v8[:, no * 8: (no + 1) * 8, kt_idx, 0:HD],
                            ps.rearrange("p (h e) -> p h e", h=8), 1.0 / WS,
                            vb_bc[:, no * 512: (no + 1) * 512]
                            .rearrange("p (h e) -> p h e", h=8),
                            op0=ALU.mult, op1=ALU.add,
                        )

                    units.append(u)
            return units

        def attention_units(q8, k8, v8, o_sink):
            """Per (head, qt-512): 8 groups of 2 key-tiles + a norm tail.

            Returns a flat closure list at group granularity for fine
            interleaving. o_sink(h, qt, o_ps) consumes the accumulated
            [VP, 512] psum (rows 0:64 raw O, row 64 softmax denom).
            """
            by_qt = []
            for qt in range(NQ):
                units = []
                by_qt.append(units)
                for h in range(NH):
                    hp, s = h // 2, h % 2
                    state = {}

                    def pv(g, h=h, state=state):
                        nc.tensor.matmul(
                            state["ops"][0:VP, :],
                            v8[:, h, 2 * g: 2 * g + 2, :],
                            state.pop(g)[:],
                            start=(g == 0), stop=(g == 7),
                            perf_mode=PM.DoubleRow,
                        )

                    def grp(g, h=h, qt=qt, hp=hp, s=s, state=state, pv=pv):
                        # scores+exp for group g; PV for group g-1 (one-group
                        # skew so a stalled PV never delays the next exp)
                        def u():
                            sc = ps_sc.tile([P, 2, 512], f32, tag="scps")
                            for j in range(2):
                                kt = 2 * g + j
                                nc.tensor.matmul(
                                    sc[:, j, :],
                                    k8[s * HD: (s + 1) * HD, hp, kt: kt + 2, :],
                                    q8[s * HD: (s + 1) * HD, hp,
                                       qt * TQ: qt * TQ + TQ]
                                    .rearrange("p (a b) -> p a b", a=2),
                                    start=True, stop=True,
                                    perf_mode=PM.DoubleRow,
                                )
                            p8 = p8p.tile([P, 2, 512], fp8, tag="p8")
                            nc.scalar.activation(p8[:], sc[:], AF.Exp,
                                                 scale=1.0 / 8.0)
                            if g == 0:
                                ovps = ps_ov.tile([P, 512], f32, tag="ovps",
                                                  name="ovps")
                                state["ops"] = ovps
                            state[g] = p8
                            if g > 0:
                                pv(g - 1)
                        return u

                    for g in range(8):
                        units.append(grp(g))

                    def norm(h=h, qt=qt, state=state, pv=pv):
                        pv(7)
                        o_sink(h, qt, state["ops"])

                    units.append(norm)
            return by_qt

        def make_o_sink(o8):
            def o_sink(h, qt, ops):
                s = h % 2
                inv = spool.tile([1, 512], bf16, tag="inv")
                with nc.allow_low_precision(reason="softmax denom recip bf16"):
                    nc.vector.reciprocal(inv[:], ops[HD: HD + 1, :])
                bc = spool.tile([HD, 512], bf16, tag="bc")
                nc.gpsimd.partition_broadcast(bc[:], inv[:])
                if s == 0:
                    nc.vector.tensor_mul(
                        o8[0:HD, h // 2, qt * 512: (qt + 1) * 512],
                        ops[0:HD, :], bc[:])
                else:
                    tmp = stg.tile([HD, 512], fp8, tag="o8tmp")
                    nc.vector.tensor_mul(tmp[:], ops[0:HD, :], bc[:])
                    nc.sync.dma_start(
                        o8[HD:P, h // 2, qt * 512: (qt + 1) * 512], tmp[:])
            return o_sink

        def oproj_units(w_sb, bname, o8, attnT, eng="dve"):
            units = []
            for n0 in range(0, TQ, 512):
                for dt in range(DT):

                    def u(dt=dt, n0=n0):
                        ps = ps_pr.tile([P, 512], f32, tag="prps")
                        for i in range(DT // 2):
                            nc.tensor.matmul(
                                ps[:],
                                w_sb[:, 2 * i: 2 * i + 2, dt * P: (dt + 1) * P],
                                o8[:, 2 * i: 2 * i + 2, n0: n0 + 512],
                                start=(i == 0),
                                stop=(i == DT // 2 - 1),
                                perf_mode=PM.DoubleRow,
                            )
                        e = eng if eng != "blk" else ("dve" if n0 == 0 else "act")
                        sink_write(attnT[:, dt, n0: n0 + 512], ps, bname, dt, e)

                    units.append(u)
            return units

        def ln_units(inT, resid_dram, get_wb, out8, out_dram=None,
                     tail_from=QC):
            """Token-major LN, one unit per 128-token chunk. Chunks at
            qc >= tail_from alternate their PSUM tile with the (then idle)
            scores pool so consecutive chunks pipeline."""
            units = []
            for qc in range(QC):

                def u(qc=qc):
                    w_bc, b_bc = get_wb()

                    def mk_tok():
                        if qc >= tail_from and qc % 2:
                            return ps_sc.tile([P, D], bf16, tag="scps",
                                              name="lntok")
                        return ps_ln.tile([P, D], bf16, tag="lntok",
                                          name="lntok")

                    tok = mk_tok()
                    for dt in range(DT):
                        nc.tensor.transpose(
                            tok[:, dt * P: (dt + 1) * P],
                            inT[:, dt, qc * P: (qc + 1) * P],
                            ident[:],
                        )
                    if resid_dram is not None:
                        s = lnp.tile([P, D], bf16, tag="lnB")
                        xq = lnp.tile([P, D], bf16, tag="lnA")
                        nc.sync.dma_start(
                            xq[:], resid_dram[qc * P: (qc + 1) * P, :]
                        )
                        nc.vector.tensor_add(s[:], xq[:], tok[:])
                    else:
                        s = tok
                    bns = spool.tile([P, 2, 6], f32, tag="bns")
                    nc.vector.bn_stats(bns[:, 0, :], s[:, 0:512])
                    nc.vector.bn_stats(bns[:, 1, :], s[:, 512:D])
                    mv = spool.tile([P, 2], f32, tag="mv")
                    nc.vector.bn_aggr(mv[:], bns[:])
                    std = spool.tile([P, 1], f32, tag="std")
                    nc.scalar.activation(std[:], mv[:, 1:2], AF.Sqrt,
                                         bias=eps_t[:])
                    rstd = spool.tile([P, 1], f32, tag="rstd")
                    nc.vector.reciprocal(rstd[:], std[:])
                    t1 = lnp.tile([P, D], bf16, tag="lnA")
                    nc.vector.scalar_tensor_tensor(
                        t1[:], s[:], mv[:, 0:1], w_bc[:],
                        op0=ALU.subtract, op1=ALU.mult,
                    )
                    if out_dram is not None:
                        o = lnp.tile([P, D], bf16, tag="lnB")
                        nc.vector.scalar_tensor_tensor(
                            o[:], t1[:], rstd[:], b_bc[:],
                            op0=ALU.mult, op1=ALU.add,
                        )
                        nc.sync.dma_start(out_dram[qc * P: (qc + 1) * P, :], o[:])
                    else:
                        nrm = lnp.tile([P, D], bf16, tag="lnnrm")
                        nc.vector.scalar_tensor_tensor(
                            nrm[:], t1[:], rstd[:], b_bc[:],
                            op0=ALU.mult, op1=ALU.add,
                        )
                        ft = mk_tok()
                        for dt in range(DT):
                            nc.tensor.transpose(
                                ft[:, dt * P: (dt + 1) * P],
                                nrm[:, dt * P: (dt + 1) * P],
                                ident[:],
                            )
                        nc.vector.tensor_copy(
                            out8[:, :, qc * P: (qc + 1) * P],
                            ft.rearrange("p (dt c) -> p dt c", dt=DT),
                        )

                units.append(u)
            return units

        def run_interleaved(primary, filler):
            k = 0
            for i, u in enumerate(primary):
                u()
                want = (i + 1) * len(filler) // len(primary)
                while k < want:
                    filler[k]()
                    k += 1
            while k < len(filler):
                filler[k]()
                k += 1

        # ------------------------------------------------------------------
        # program
        # ------------------------------------------------------------------
        # Phase 1 (prefix kept minimal: attention-1 needs only Kf + Qt +
        # Vf-block0; remaining Vf blocks become attention fillers).
        loaders_f = [x_block_loader(xfT_d, bi) for bi in range(4)]
        loaders_f[0]()
        g_kf = weight_loader("kf")
        g_kf()
        load_bias_cols()
        g_qt = weight_loader("qt")
        for bi in range(4):
            for u in featmaj_units(
                g_kf, loaders_f[bi],
                qk_sink_k(k8_f, bi * 4, "kf", eng="alt"),
            ):
                u()
        for qt in range(NQ):
            get_x = x_block_loader(xtT_d, qt)
            for u in featmaj_units(
                g_qt, get_x,
                qk_sink("qt", lambda dt, qt=qt: q8_t[:, dt, qt * TQ:
                                                     qt * TQ + 512],
                        eng="alt"),
            ):
                u()
        g_vf = weight_loader("vf")
        vb_f = row_bcast(br_d["vf"][:, :], "vbc1")
        loaders_fv = [x_block_loader(xfT_d, bi) for bi in range(4)]
        for u in v_units(g_vf, vb_f, loaders_fv[0], 0, v8_f):
            u()

        # Phase 2: attention-1 || Vf(1-3)/Kt+Vt/Qf.  Weight loads are lazy
        # (the load lands in the SP stream at its consumer's position), and
        # Kt/Vt units interleave per block so they share one x load within
        # the xs pool window.
        g_kt = weight_loader("kt")
        g_vt = weight_loader("vt")
        g_qf = weight_loader("qf")
        vb_t = row_bcast(br_d["vt"][:, :], "vbc2")
        loaders_t = [x_block_loader(xtT_d, bi) for bi in range(4)]
        fillers = []
        for bi in range(1, 4):
            fillers += v_units(g_vf, vb_f, loaders_fv[bi], bi * 512, v8_f)
        fillers.insert(len(fillers) - 2, lambda: (g_kt(), None)[1])
        fillers.insert(len(fillers) - 1, lambda: (g_vt(), None)[1])
        for bi in range(4):
            fillers += featmaj_units(
                g_kt, loaders_t[bi], qk_sink_k(k8_t, bi * 4, "kt"))
            fillers += v_units(g_vt, vb_t, loaders_t[bi], bi * 512, v8_t)
        fillers.insert(len(fillers) - 2, lambda: (g_qf(), None)[1])
        for qt in range(NQ):
            get_x = x_block_loader(xfT_d, qt)
            fillers += featmaj_units(
                g_qf, get_x,
                qk_sink("qf", lambda dt, qt=qt: q8_f[:, dt, qt * TQ:
                                                     qt * TQ + 512]),
            )
        a1_qt0, a1_qt1 = attention_units(q8_t, k8_f, v8_f, make_o_sink(o8_t))
        nhalf = len(fillers) // 2
        run_interleaved(a1_qt0, fillers[:nhalf])
        run_interleaved(a1_qt1, fillers[nhalf:])

        # Phase 3: attention-2. qt0 half || O-proj(t)+LN(t); qt1 half ||
        # block-0 endgame (O-proj(f) blk0 + LN(f) 0-3).
        w_ot = load_weight("ot")
        attnT_t = res.tile([P, DT, TQ], bf16, name="attnT_t", tag="big")
        fused8_t = res.tile([P, DT, TQ], fp8, name="fused8_t", tag="qt8")
        o8_f = res.tile([P, DT, TQ], fp8, name="o8_f", tag="k8f")
        lnt_wb = row_bcast(ln_d["lnt_w"].rearrange("(a d) -> a d", a=1), "lnw1")
        lnt_bb = row_bcast(ln_d["lnt_b"].rearrange("(a d) -> a d", a=1), "lnb1")
        oprojA = oproj_units(w_ot, "ot", o8_t, attnT_t)
        lnA = ln_units(attnT_t, xtq_d, lambda: (lnt_wb, lnt_bb), fused8_t)
        fillers2 = oprojA[:8]
        for i in range(4):
            fillers2.append(oprojA[8 + 2 * i])
            fillers2.append(oprojA[9 + 2 * i])
            fillers2.append(lnA[i])
        fillers2 += lnA[4:]

        w_of = load_weight("of")
        attnT_f = res.tile([P, DT, TQ], bf16, name="attnT_f", tag="big")
        fused8_f = res.tile([P, DT, TQ], fp8, name="fused8_f", tag="ff8")
        lnf_wb = row_bcast(ln_d["lnf_w"].rearrange("(a d) -> a d", a=1), "lnw2")
        lnf_bb = row_bcast(ln_d["lnf_b"].rearrange("(a d) -> a d", a=1), "lnb2")
        oprojB = oproj_units(w_of, "of", o8_f, attnT_f, eng="blk")
        lnB = ln_units(attnT_f, xfq_d, lambda: (lnf_wb, lnf_bb), fused8_f,
                       tail_from=4)

        a2_qt0, a2_qt1 = attention_units(q8_f, k8_t, v8_t, make_o_sink(o8_f))
        run_interleaved(a2_qt0, fillers2)
        fillers3 = oprojB[:8] + lnB[:4]
        run_interleaved(a2_qt1, fillers3)

        # Phase 4 tail: O-proj(f) blk1, LN(f) 4-7, fus1, fus2, LN(fus)
        g_f1a = weight_loader("f1", kts=(0, DT))
        g_f1b = weight_loader("f1", kts=(DT, 2 * DT))
        h8 = res.tile([P, DT, TQ], fp8, name="h8", tag="h8t")

        def fus1_units(n0):
            units = []
            for dt in range(DT):

                def u(dt=dt, n0=n0):
                    w_f1a = g_f1a()
                    w_f1b = g_f1b()
                    ps = ps_pr.tile([P, 512], f32, tag="prps")
                    for i in range(DT // 2):
                        nc.tensor.matmul(
                            ps[:],
                            w_f1a[:, 2 * i: 2 * i + 2, dt * P: (dt + 1) * P],
                            fused8_t[:, 2 * i: 2 * i + 2, n0: n0 + 512],
                            start=(i == 0), stop=False,
                            perf_mode=PM.DoubleRow,
                        )
                    for i in range(DT // 2):
                        nc.tensor.matmul(
                            ps[:],
                            w_f1b[:, 2 * i: 2 * i + 2, dt * P: (dt + 1) * P],
                            fused8_f[:, 2 * i: 2 * i + 2, n0: n0 + 512],
                            start=False, stop=(i == DT // 2 - 1),
                            perf_mode=PM.DoubleRow,
                        )
                    nc.scalar.activation(
                        h8[:, dt, n0: n0 + 512], ps[:], AF.Gelu,
                        bias=bias_col["f1"][:, dt: dt + 1], scale=1.0 / WS,
                    )

                units.append(u)
            return units

        g_f2 = weight_loader("f2")
        o2T = res.tile([P, DT, TQ], bf16, name="o2T", tag="v8f")
        lnu_state = {}

        def get_lnu():
            # lazy: reuses the lnt broadcast tags once lnA is finished
            if "w" not in lnu_state:
                lnu_state["w"] = row_bcast(
                    ln_d["lnu_w"].rearrange("(a d) -> a d", a=1), "lnw1")
                lnu_state["b"] = row_bcast(
                    ln_d["lnu_b"].rearrange("(a d) -> a d", a=1), "lnb1")
            return lnu_state["w"], lnu_state["b"]

        lnU = ln_units(o2T, None, get_lnu, None, out_dram=out_d, tail_from=0)

        def fus2_units(n0, eng):
            return featmaj_units(
                g_f2, lambda n0=n0: h8[:, :, n0: n0 + 512],
                qk_sink("f2", lambda dt, n0=n0: o2T[:, dt, n0: n0 + 512],
                        eng=eng),
            )

        run_interleaved(oprojB[8:],
                        [lnB[4], lambda: (g_f1a(), None)[1],
                         lnB[5], lambda: (g_f1b(), None)[1]])
        run_interleaved(fus1_units(0),
                        [lnB[6], lambda: (g_f2(), None)[1], lnB[7]])
        for u in fus1_units(512):
            u()
        for u in fus2_units(0, "dve"):
            u()
        run_interleaved(fus2_units(512, "act"), lnU[:4])
        for u in lnU[4:]:
            u()

    nc.compile()
    return nc


# ---------------------------------------------------------------------------
# host side
# ---------------------------------------------------------------------------
_CACHE = {}


def _get_nc():
    if "nc" not in _CACHE:
        _CACHE["nc"] = _build_nc()
    return _CACHE["nc"]


def _make_in_maps(inputs):
    import ml_dtypes

    F8 = ml_dtypes.float8_e4m3

    def q8(a):
        return np.clip(a, -240.0, 240.0).astype(F8)

    def wshuf(w):
        # [din, dout] -> partition-major [128, din/128, dout], x32, fp8
        w = np.asarray(w, np.float32) * WS
        nkt = w.shape[0] // P
        return q8(np.ascontiguousarray(
            w.reshape(nkt, P, w.shape[1]).transpose(1, 0, 2)))

    def xshuf(xT):
        # [D, T] -> [T/512 blocks, 128, DT, 512], fp8
        return q8(np.ascontiguousarray(
            xT.reshape(DT, P, T // 512, 512).transpose(2, 1, 0, 3)))

    t = np.asarray(inputs["temporal_tokens"], np.float32)
    f = np.asarray(inputs["feature_tokens"], np.float32)

    def bshuf(b):
        return np.ascontiguousarray(
            np.asarray(b, np.float32).reshape(DT, P).T)

    shared = {}
    for n in _WNAMES:
        shared[f"w_{n}"] = wshuf(inputs[f"{n}_w"])
        shared[f"b_{n}"] = bshuf(inputs[f"{n}_b"])
    shared["w_f1"] = wshuf(inputs["fus1_w"])
    shared["b_f1"] = bshuf(inputs["fus1_b"])
    shared["w_f2"] = wshuf(inputs["fus2_w"])
    shared["b_f2"] = bshuf(inputs["fus2_b"])
    bf16 = ml_dtypes.bfloat16
    for n in ["vf", "vt"]:
        shared[f"br_{n}"] = np.ascontiguousarray(
            np.asarray(inputs[f"{n}_b"], np.float32).reshape(1, D)
        ).astype(bf16)
    for src, dst in [
        ("ln_t_w", "lnt_w"), ("ln_t_b", "lnt_b"),
        ("ln_f_w", "lnf_w"), ("ln_f_b", "lnf_b"),
        ("ln_fus_w", "lnu_w"), ("ln_fus_b", "lnu_b"),
    ]:
        shared[dst] = np.ascontiguousarray(inputs[src]).astype(bf16)

    in_maps = []
    for c in range(8):
        b, half = divmod(c, 2)
        r0 = half * TQ
        xt = t[b]
        xf = f[b]
        # query rows first, remaining rows after (K/V order is irrelevant)
        perm = np.concatenate([np.arange(r0, T), np.arange(0, r0)])
        m = dict(shared)
        m["xtT"] = xshuf(xt[perm].T)
        m["xfT"] = xshuf(xf[perm].T)
        m["xtq"] = np.ascontiguousarray(xt[r0: r0 + TQ]).astype(bf16)
        m["xfq"] = np.ascontiguousarray(xf[r0: r0 + TQ]).astype(bf16)
        in_maps.append(m)
    return in_maps


def kernel(**inputs):
    try:
        import jax

        jax.config.update("jax_compilation_cache_dir", "/tmp/jaxcache")
        jax.config.update("jax_persistent_cache_min_entry_size_bytes", -1)
        jax.config.update("jax_persistent_cache_min_compile_time_secs", 0.0)
    except Exception:
        pass
    from concourse.bass_utils import run_bass_kernel_spmd

    nc = _get_nc()
    in_maps = _make_in_maps(inputs)
    res = run_bass_kernel_spmd(nc, in_maps, list(range(8)))
    B = 4
    out = np.empty((B, T, D), np.float32)
    for c in range(8):
        b, half = divmod(c, 2)
        out[b, half * TQ: (half + 1) * TQ] = np.asarray(
            res.results[c]["out"]).astype(np.float32)
    return out
